# revision 12
# baseline (speedup 1.0000x reference)
"""HRR binding self-attention kernel for 8 trn2 NeuronCores.

Math: out = irfft(c * rfft(x) * cumsum_s(rfft(x))) @ w_out.T  with c = queries*keyvalues.
rfft is linear so cumsum commutes with it: ONE forward DFT of x, prefix sum in the
frequency domain.  irfft and the output Linear fuse into a single matmul:
out = qv^T @ GW with GW = (c*G) @ w_out.T precomputed on host (c folded in too).

Sharding: 8 shards = (batch b in 0..3) x (seq half h in 0..1), 2048 tokens each.
h=1 shards get the first half's contribution as an initial carry, computed on host
as rfft(x[b, :2048].sum(0)) (negligible).

Packed real spectrum (2048 rows): rows 0..1024 = Re[0..1024], rows 1025..2047 =
Im[1..1023].  Chunked [P=128, chunk=16]: chunk j partition p = Re[128j+p],
chunk 8+j partition p = Im[128j+p] (chunk 8 partition 0 = Nyquist Re[1024]).
Complex multiplies pair chunk j with chunk 8+j on equal partitions, 2-row fixup
for DC/Nyquist.

Per-core pipeline, one pass over 512-token slabs (all matmuls bf16, fp32 PSUM):
  - DFT: CS-slice stationary, x^T slab moving -> freq-major spectrum in PSUM
  - cumsum over tokens: DVE tensor_tensor_scan straight from PSUM (f32 state),
    per-partition carry chained across slabs
  - complex multiply (DVE) -> qv bf16 in SBUF
  - GW matmul: qv-slice stationary, GW moving -> token-major out in PSUM,
    drained by the scalar engine and DMA'd; GW runs one slab behind the DFT
    so the PE never waits on the DVE tail.
"""

import sys

sys.path.insert(0, "/opt/trn_rl_repo")

import numpy as np
import ml_dtypes

import concourse.bass as bass
import concourse.bacc as bacc
import concourse.mybir as mybir
from concourse.tile import TileContext
from concourse.bass_utils import run_bass_kernel_spmd

BF16 = mybir.dt.bfloat16
F32 = mybir.dt.float32
ALU = mybir.AluOpType

P = 128
D = 2048  # model dims
T = 2048  # tokens per shard
ND = D // P  # 16 d-chunks
NPF = 16  # packed-frequency chunks
TS = 512  # tokens per slab
NSL = T // TS  # slabs
NB = 4  # batch
NS = 4096  # full seq

bf16 = ml_dtypes.bfloat16

_CACHE = {}


def _build_nc(reps: int = 1):
    nc = bacc.Bacc("TRN2", target_bir_lowering=False, debug=False, num_devices=8)
    xS = nc.dram_tensor("xS", [NSL, P, ND, TS], BF16, kind="ExternalInput")
    # CS repacked by frequency pair: CS[j] holds DFT cols for packed chunks (j, j+8)
    CS = nc.dram_tensor("CS", [8, P, ND, 2 * P], BF16, kind="ExternalInput")
    GW = nc.dram_tensor("GW", [P, NPF, D], BF16, kind="ExternalInput")
    C0 = nc.dram_tensor("C0", [P, NPF], F32, kind="ExternalInput")
    out = nc.dram_tensor("out", [T, D], F32, kind="ExternalOutput")

    with TileContext(nc) as tc:
        import contextlib

        loop_ctx = tc.For_i(0, reps, 1) if reps > 1 else contextlib.nullcontext()
        with loop_ctx:
            _body(nc, tc, xS, CS, GW, C0, out)
    nc.finalize()
    return nc


def _body(nc, tc, xS, CS, GW, C0, out):
    with (
        tc.tile_pool(name="const", bufs=1) as cpool,
        tc.tile_pool(name="xt", bufs=2) as xpool,
        tc.tile_pool(name="qv", bufs=2) as qvpool,
        tc.tile_pool(name="s", bufs=2) as spool,
        tc.tile_pool(name="tmp", bufs=1) as tpool,
        tc.tile_pool(name="osb", bufs=2) as opool,
        tc.tile_pool(name="psD", bufs=2, space="PSUM") as psD,
        tc.tile_pool(name="psG", bufs=2, space="PSUM") as psG,
    ):
        # xt slab 0 first on the SP ring so the first DFT isn't stuck behind
        # constant loads; constants go on the Act ring (per-pair CS chunks so
        # pair j only waits on its own 1MB).
        xt0 = xpool.tile([P, ND, TS], BF16, tag="xt")
        nc.sync.dma_start(xt0[:], xS[0])
        cs_sb = cpool.tile([P, ND, 16, P], BF16)
        for j in range(8):
            nc.scalar.dma_start(cs_sb[:, :, 2 * j : 2 * j + 2, :], CS[j])
        carry = cpool.tile([P, NPF], F32)
        nc.scalar.dma_start(carry[:], C0[:])
        gw_sb = cpool.tile([P, NPF, D], BF16)
        for g in range(4):
            nc.scalar.dma_start(gw_sb[:, 4 * g : 4 * (g + 1), :], GW[:, 4 * g : 4 * (g + 1), :])
        zeros = cpool.tile([P, TS], BF16)
        nc.vector.memset(zeros[:], 0.0)

        qv_prev = None
        for s in range(NSL):
            if s == 0:
                xt = xt0
            else:
                xt = xpool.tile([P, ND, TS], BF16, tag="xt")
                nc.sync.dma_start(xt[:], xS[s])
            qv = qvpool.tile([P, NPF, TS], BF16, tag="qv")

            for j in range(8):
                ps = psD.tile([P, 2, TS], F32, tag="psD")
                for d in range(ND):
                    nc.tensor.matmul(
                        ps[:, 0, :],
                        cs_sb[:, d, 2 * j, :],
                        xt[:, d, :],
                        start=(d == 0),
                        stop=(d == ND - 1),
                    )
                    nc.tensor.matmul(
                        ps[:, 1, :],
                        cs_sb[:, d, 2 * j + 1, :],
                        xt[:, d, :],
                        start=(d == 0),
                        stop=(d == ND - 1),
                    )
                S = spool.tile([P, 2, TS], BF16, tag="S")
                nc.vector.tensor_tensor_scan(
                    S[:, 0, :], ps[:, 0, :], zeros[:], carry[:, j : j + 1],
                    ALU.add, ALU.add,
                )
                nc.vector.tensor_tensor_scan(
                    S[:, 1, :], ps[:, 1, :], zeros[:], carry[:, j + 8 : j + 9],
                    ALU.add, ALU.add,
                )
                nc.gpsimd.tensor_copy(carry[:, j : j + 1], S[:, 0, TS - 1 : TS])
                nc.gpsimd.tensor_copy(carry[:, j + 8 : j + 9], S[:, 1, TS - 1 : TS])

                t1 = tpool.tile([P, TS], F32, tag="t1")
                t2 = tpool.tile([P, TS], F32, tag="t2")
                nc.vector.tensor_mul(t1[:], ps[:, 0, :], S[:, 0, :])
                nc.vector.tensor_mul(t2[:], ps[:, 1, :], S[:, 1, :])
                nc.vector.tensor_sub(qv[:, j, :], t1[:], t2[:])
                t3 = tpool.tile([P, TS], F32, tag="t1")
                t4 = tpool.tile([P, TS], F32, tag="t2")
                nc.vector.tensor_mul(t3[:], ps[:, 0, :], S[:, 1, :])
                nc.vector.tensor_mul(t4[:], ps[:, 1, :], S[:, 0, :])
                nc.vector.tensor_add(qv[:, j + 8, :], t3[:], t4[:])
                if j == 0:
                    # DC (chunk 0 row 0) and Nyquist (chunk 8 row 0) are purely real
                    nc.vector.tensor_mul(qv[0:1, 0, :], ps[0:1, 0, :], S[0:1, 0, :])
                    nc.vector.tensor_mul(qv[0:1, 8, :], ps[0:1, 1, :], S[0:1, 1, :])

            if qv_prev is not None:
                _gw_block(nc, tc, gw_sb, qv_prev, out, s - 1, opool, psG)
            qv_prev = qv
        _gw_block(nc, tc, gw_sb, qv_prev, out, NSL - 1, opool, psG)


def _gw_block(nc, tc, gw_sb, qv, out, s, opool, psG):
    """out[s*TS + tg*128 + t, e] = sum_r qv[r, tg*128+t] * GW[r, e]"""
    for tg in range(TS // P):
        for eh in range(2):  # e half: out cols [eh*1024, (eh+1)*1024)
            ps = psG.tile([P, 2, 512], F32, tag="psG")
            for pf in range(NPF):
                for e2 in range(2):
                    e = 2 * eh + e2
                    nc.tensor.matmul(
                        ps[:, e2, :],
                        qv[:, pf, tg * P : (tg + 1) * P],
                        gw_sb[:, pf, e * 512 : (e + 1) * 512],
                        start=(pf == 0),
                        stop=(pf == NPF - 1),
                    )
            r0 = s * TS + tg * P
            for e2 in range(2):
                osb = opool.tile([P, 512], F32, tag="osb")
                nc.scalar.copy(osb[:], ps[:, e2, :])
                e = 2 * eh + e2
                nc.sync.dma_start(out[r0 : r0 + P, e * 512 : (e + 1) * 512], osb[:])


def _chunked(m):
    """[rows, cols] -> [P, rows//P, cols] with row r at [r % P, r // P]."""
    r, c = m.shape
    return np.ascontiguousarray(m.reshape(r // P, P, c).transpose(1, 0, 2))


def _pack_spec(re, im):
    """re[1025], im[1025] -> packed [2048]: re[0..1024] then im[1..1023]."""
    return np.concatenate([re, im[1:1024]])


def _constants():
    if "consts" in _CACHE:
        return _CACHE["consts"]
    d = np.arange(D, dtype=np.float64)
    f = np.arange(D // 2 + 1, dtype=np.float64)
    ang = 2.0 * np.pi / D * np.outer(d, f)  # [D, 1025]
    cos, sin = np.cos(ang), np.sin(ang)
    CSf = np.concatenate([cos, -sin[:, 1:1024]], axis=1)  # [D, D]
    alpha = np.full(1025, 2.0)
    alpha[0] = alpha[1024] = 1.0
    Gf = np.concatenate(
        [(alpha[:, None] * cos.T) / D, (-2.0 * sin[:, 1:1024].T) / D], axis=0
    )  # [D packed, D]
    CSc = _chunked(CSf.astype(np.float32))  # [P, ND, D packed cols]
    # repack by frequency pair: CS2[j] = cols of packed chunks (j, j+8)
    CS2 = np.stack(
        [
            np.concatenate(
                [CSc[:, :, j * P : (j + 1) * P], CSc[:, :, (j + 8) * P : (j + 9) * P]],
                axis=2,
            )
            for j in range(8)
        ]
    )  # [8, P, ND, 2P]
    consts = {"CS": np.ascontiguousarray(CS2).astype(bf16), "Gf": Gf}
    _CACHE["consts"] = consts
    return consts


def prepare_in_maps(x, queries, keyvalues, w_out):
    x = np.asarray(x, dtype=np.float32)
    queries = np.asarray(queries, dtype=np.float32)
    keyvalues = np.asarray(keyvalues, dtype=np.float32)
    w_out = np.asarray(w_out, dtype=np.float32)
    consts = _constants()

    c = (queries * keyvalues).reshape(-1)  # [1025]
    c_packed = _pack_spec(c, c)  # [2048]
    GWf = (c_packed[:, None] * consts["Gf"]).astype(np.float32) @ np.ascontiguousarray(
        w_out.T
    )  # [D packed, D out]
    GWc = _chunked(GWf).astype(bf16)

    in_maps = []
    for b in range(NB):
        for h in range(2):
            xs = x[b, h * T : (h + 1) * T]  # [T, D]
            xT3 = _chunked(np.ascontiguousarray(xs.T))  # [P, ND, T]
            xSc = np.ascontiguousarray(
                xT3.reshape(P, ND, NSL, TS).transpose(2, 0, 1, 3)
            ).astype(bf16)
            if h == 0:
                c0 = np.zeros((P, NPF), np.float32)
            else:
                F = np.fft.rfft(x[b, :T].sum(axis=0).astype(np.float64))
                c0 = _chunked(
                    _pack_spec(F.real, F.imag).astype(np.float32)[:, None]
                )[:, :, 0]
            in_maps.append(
                {
                    "xS": xSc,
                    "CS": consts["CS"],
                    "GW": GWc,
                    "C0": np.ascontiguousarray(c0),
                }
            )
    return in_maps


def kernel(x, queries, keyvalues, w_out):
    if "nc" not in _CACHE:
        _CACHE["nc"] = _build_nc()
    nc = _CACHE["nc"]
    in_maps = prepare_in_maps(x, queries, keyvalues, w_out)
    res = run_bass_kernel_spmd(nc, in_maps, core_ids=list(range(8)))
    y = np.empty((NB, NS, D), np.float32)
    for i in range(8):
        b, h = i // 2, i % 2
        y[b, h * T : (h + 1) * T] = res.results[i]["out"]
    return y


# revision 13
# speedup vs baseline: 1.3563x; 1.3563x over previous
"""HRR binding self-attention kernel for 8 trn2 NeuronCores.

Math: out = irfft(c * rfft(x) * cumsum_s(rfft(x))) @ w_out.T  with c = queries*keyvalues.
rfft is linear so cumsum commutes with it; the prefix sum runs in the frequency
domain.  irfft and the output Linear fuse into one matmul: out = qv^T @ GW with
GW = (c*G) @ w_out.T precomputed on host.

The forward rfft is radix-2 split: X[k] = E[k] + W^k O[k] with E,O = packed
rfft_1024 of even/odd samples — two 1024-wide DFT matmuls (half the PE work of
a direct 2048 DFT).  The upper half spectrum (k>512) needs partition-mirrored
E/O rows: cheap permutation matmuls (reversal + p0-pick stationaries) provide
them; conjugation signs and the W^k twiddle fold into per-partition scalars of
scalar_tensor_tensor combine ops on the DVE.

Packed spectra (1024 rows for E/O, 2048 for X): Re[0..N/2] then Im[1..N/2-1];
chunked [P=128 x chunks], chunk j pairs with chunk j+nchunks/2 on equal
partitions for complex ops; DC/Nyquist ride partition 0 with fixups.

Sharding: 8 shards = (batch b in 0..3) x (seq half h in 0..1), 2048 tokens
each; h=1 shards get the first half's spectrum sum as initial cumsum carry
(host rfft of x[b,:2048].sum(0)).

Per-core pipeline over 512-token slabs (all matmuls bf16, fp32 PSUM):
  EO-DFT -> ACT drains to SBUF -> mirror matmuls -> DVE combine -> DVE
  tensor_tensor_scan (cumsum, carry chained across slabs) -> DVE complex
  multiply -> qv bf16 -> GW matmul (one slab behind, keeps PE dense) ->
  ACT drain -> DMA out.
"""

import sys

sys.path.insert(0, "/opt/trn_rl_repo")

import numpy as np
import ml_dtypes

import concourse.bass as bass
import concourse.bacc as bacc
import concourse.mybir as mybir
from concourse.tile import TileContext
from concourse.bass_utils import run_bass_kernel_spmd

BF16 = mybir.dt.bfloat16
F32 = mybir.dt.float32
ALU = mybir.AluOpType

P = 128
D = 2048  # model dims
T = 2048  # tokens per shard
ND = D // P  # 16 chunks of x (8 even + 8 odd)
NPF = 16  # packed-frequency chunks of X
TS = 512  # tokens per slab
NSL = T // TS  # slabs
NB = 4  # batch
NS = 4096  # full seq

bf16 = ml_dtypes.bfloat16

_CACHE = {}


def _build_nc(reps: int = 1):
    nc = bacc.Bacc("TRN2", target_bir_lowering=False, debug=False, num_devices=8)
    # xS chunks 0..7 = even samples chunked, 8..15 = odd samples chunked
    xS = nc.dram_tensor("xS", [NSL, P, ND, TS], BF16, kind="ExternalInput")
    CS = nc.dram_tensor("CS", [P, 8, 1024], BF16, kind="ExternalInput")  # 1024-DFT
    RR = nc.dram_tensor("RR", [P, 4, P], BF16, kind="ExternalInput")  # Rrev,R00,-Rrev,-R00
    CA = nc.dram_tensor("CA", [P, NPF], F32, kind="ExternalInput")
    CB = nc.dram_tensor("CB", [P, NPF], F32, kind="ExternalInput")
    GW = nc.dram_tensor("GW", [P, NPF, D], BF16, kind="ExternalInput")
    C0 = nc.dram_tensor("C0", [P, NPF], F32, kind="ExternalInput")
    out = nc.dram_tensor("out", [T, D], F32, kind="ExternalOutput")

    with TileContext(nc) as tc:
        import contextlib

        loop_ctx = tc.For_i(0, reps, 1) if reps > 1 else contextlib.nullcontext()
        with loop_ctx:
            _body(nc, tc, xS, CS, RR, CA, CB, GW, C0, out)
    nc.finalize()
    return nc


def _body(nc, tc, xS, CS, RR, CA, CB, GW, C0, out):
    with (
        tc.tile_pool(name="const", bufs=1) as cpool,
        tc.tile_pool(name="xt", bufs=2) as xpool,
        tc.tile_pool(name="eo", bufs=1) as eopool,
        tc.tile_pool(name="mir", bufs=1) as mpool,
        tc.tile_pool(name="X", bufs=2) as Xpool,
        tc.tile_pool(name="qv", bufs=2) as qvpool,
        tc.tile_pool(name="s", bufs=2) as spool,
        tc.tile_pool(name="tmp", bufs=1) as tpool,
        tc.tile_pool(name="osb", bufs=2) as opool,
        tc.tile_pool(name="psEO", bufs=2, space="PSUM") as psEO,
        tc.tile_pool(name="psM", bufs=2, space="PSUM") as psM,
        tc.tile_pool(name="psG", bufs=2, space="PSUM") as psG,
    ):
        # slab 0 input first on the ring so the first DFT starts ASAP
        xt0 = xpool.tile([P, ND, TS], BF16, tag="xt")
        nc.sync.dma_start(xt0[:], xS[0])
        cs_sb = cpool.tile([P, 8, 1024], BF16)
        nc.sync.dma_start(cs_sb[:], CS[:])
        rr_sb = cpool.tile([P, 4, P], BF16)
        nc.sync.dma_start(rr_sb[:], RR[:])
        ca_sb = cpool.tile([P, NPF], F32)
        nc.sync.dma_start(ca_sb[:], CA[:])
        cb_sb = cpool.tile([P, NPF], F32)
        nc.sync.dma_start(cb_sb[:], CB[:])
        carry = cpool.tile([P, NPF], F32)
        nc.sync.dma_start(carry[:], C0[:])
        gw_sb = cpool.tile([P, NPF, D], BF16)
        for g in range(4):
            nc.sync.dma_start(
                gw_sb[:, 4 * g : 4 * (g + 1), :], GW[:, 4 * g : 4 * (g + 1), :]
            )
        zeros = cpool.tile([P, TS], BF16)
        nc.vector.memset(zeros[:], 0.0)

        qv_prev = None
        for s in range(NSL):
            if s == 0:
                xt = xt0
            else:
                xt = xpool.tile([P, ND, TS], BF16, tag="xt")
                nc.sync.dma_start(xt[:], xS[s])

            # ---- EO-DFT: 16 packed output chunks (8 E + 8 O) ----
            eo = eopool.tile([P, NPF, TS], BF16, tag="eo")
            for oc in range(NPF):
                base = 0 if oc < 8 else 8
                col = oc % 8
                ps = psEO.tile([P, TS], F32, tag="psEO")
                for d in range(8):
                    nc.tensor.matmul(
                        ps[:],
                        cs_sb[:, d, col * P : (col + 1) * P],
                        xt[:, base + d, :],
                        start=(d == 0),
                        stop=(d == 7),
                    )
                nc.scalar.copy(eo[:, oc, :], ps[:])

            # ---- mirrors for the upper half (chunks c=4..7) ----
            # M layout: 0..3 EreM, 4..7 OreM, 8..11 OimM, 12..15 -EimM
            mir = mpool.tile([P, NPF, TS], BF16, tag="mir")
            for i, c in enumerate(range(4, 8)):
                specs = [
                    (i, [(0, 7 - c), (1, 8 - c)]),               # EreM: Rrev,R00 on E
                    (4 + i, [(0, 8 + 7 - c), (1, 8 + 8 - c)]),   # OreM on O
                    (8 + i, [(0, 8 + 11 - c)] + ([(1, 8 + 12 - c)] if c > 4 else [])),
                    (12 + i, [(2, 11 - c)] + ([(3, 12 - c)] if c > 4 else [])),
                ]
                for mslot, terms in specs:
                    pm = psM.tile([P, TS], F32, tag="psM")
                    for ti, (rsel, ech) in enumerate(terms):
                        nc.tensor.matmul(
                            pm[:],
                            rr_sb[:, rsel, :],
                            eo[:, ech, :],
                            start=(ti == 0),
                            stop=(ti == len(terms) - 1),
                        )
                    nc.scalar.copy(mir[:, mslot, :], pm[:])

            # ---- GW block one slab behind (keeps PE dense while DVE works) ----
            if qv_prev is not None:
                _gw_block(nc, tc, gw_sb, qv_prev, out, s - 1, opool, psG)

            # ---- combine + scan + complex multiply, per frequency pair ----
            qv = qvpool.tile([P, NPF, TS], BF16, tag="qv")
            for j in range(8):
                X = Xpool.tile([P, 2, TS], BF16, tag="X")
                tA = tpool.tile([P, TS], BF16, tag="tA")
                if j <= 3:
                    nc.vector.scalar_tensor_tensor(
                        tA[:], eo[:, 8 + j, :], ca_sb[:, j : j + 1], eo[:, j, :],
                        ALU.mult, ALU.add,
                    )
                    nc.vector.scalar_tensor_tensor(
                        X[:, 0, :], eo[:, 12 + j, :], cb_sb[:, j : j + 1], tA[:],
                        ALU.mult, ALU.add,
                    )
                    tB = tpool.tile([P, TS], BF16, tag="tB")
                    nc.vector.scalar_tensor_tensor(
                        tB[:], eo[:, 12 + j, :], ca_sb[:, 8 + j : 9 + j], eo[:, 4 + j, :],
                        ALU.mult, ALU.add,
                    )
                    nc.vector.scalar_tensor_tensor(
                        X[:, 1, :], eo[:, 8 + j, :], cb_sb[:, 8 + j : 9 + j], tB[:],
                        ALU.mult, ALU.add,
                    )
                else:
                    i = j - 4
                    nc.vector.scalar_tensor_tensor(
                        tA[:], mir[:, 4 + i, :], ca_sb[:, j : j + 1], mir[:, i, :],
                        ALU.mult, ALU.add,
                    )
                    nc.vector.scalar_tensor_tensor(
                        X[:, 0, :], mir[:, 8 + i, :], cb_sb[:, j : j + 1], tA[:],
                        ALU.mult, ALU.add,
                    )
                    tB = tpool.tile([P, TS], BF16, tag="tB")
                    nc.vector.scalar_tensor_tensor(
                        tB[:], mir[:, 8 + i, :], ca_sb[:, 8 + j : 9 + j], mir[:, 12 + i, :],
                        ALU.mult, ALU.add,
                    )
                    nc.vector.scalar_tensor_tensor(
                        X[:, 1, :], mir[:, 4 + i, :], cb_sb[:, 8 + j : 9 + j], tB[:],
                        ALU.mult, ALU.add,
                    )
                if j == 0:
                    # X row 1024 (chunk 8, p0) is Nyquist: Re X[1024] = E[0] - O[0]
                    nc.vector.tensor_sub(
                        X[0:1, 1, :], eo[0:1, 0, :], eo[0:1, 8, :]
                    )

                S = spool.tile([P, 2, TS], BF16, tag="S")
                nc.vector.tensor_tensor_scan(
                    S[:, 0, :], X[:, 0, :], zeros[:], carry[:, j : j + 1],
                    ALU.add, ALU.add,
                )
                nc.vector.tensor_tensor_scan(
                    S[:, 1, :], X[:, 1, :], zeros[:], carry[:, j + 8 : j + 9],
                    ALU.add, ALU.add,
                )
                nc.vector.tensor_copy(carry[:, j : j + 1], S[:, 0, TS - 1 : TS])
                nc.vector.tensor_copy(carry[:, j + 8 : j + 9], S[:, 1, TS - 1 : TS])

                t1 = tpool.tile([P, TS], F32, tag="t1")
                t2 = tpool.tile([P, TS], F32, tag="t2")
                nc.vector.tensor_mul(t1[:], X[:, 0, :], S[:, 0, :])
                nc.vector.tensor_mul(t2[:], X[:, 1, :], S[:, 1, :])
                nc.vector.tensor_sub(qv[:, j, :], t1[:], t2[:])
                t3 = tpool.tile([P, TS], F32, tag="t1")
                t4 = tpool.tile([P, TS], F32, tag="t2")
                nc.vector.tensor_mul(t3[:], X[:, 0, :], S[:, 1, :])
                nc.vector.tensor_mul(t4[:], X[:, 1, :], S[:, 0, :])
                nc.vector.tensor_add(qv[:, j + 8, :], t3[:], t4[:])
                if j == 0:
                    # DC (chunk 0 p0) and Nyquist (chunk 8 p0) are purely real
                    nc.vector.tensor_mul(qv[0:1, 0, :], X[0:1, 0, :], S[0:1, 0, :])
                    nc.vector.tensor_mul(qv[0:1, 8, :], X[0:1, 1, :], S[0:1, 1, :])

            qv_prev = qv
        _gw_block(nc, tc, gw_sb, qv_prev, out, NSL - 1, opool, psG)


def _gw_block(nc, tc, gw_sb, qv, out, s, opool, psG):
    """out[s*TS + tg*128 + t, e] = sum_r qv[r, tg*128+t] * GW[r, e]"""
    for tg in range(TS // P):
        for eh in range(2):
            ps = psG.tile([P, 2, 512], F32, tag="psG")
            for pf in range(NPF):
                for e2 in range(2):
                    e = 2 * eh + e2
                    nc.tensor.matmul(
                        ps[:, e2, :],
                        qv[:, pf, tg * P : (tg + 1) * P],
                        gw_sb[:, pf, e * 512 : (e + 1) * 512],
                        start=(pf == 0),
                        stop=(pf == NPF - 1),
                    )
            r0 = s * TS + tg * P
            for e2 in range(2):
                osb = opool.tile([P, 512], F32, tag="osb")
                nc.scalar.copy(osb[:], ps[:, e2, :])
                e = 2 * eh + e2
                nc.sync.dma_start(out[r0 : r0 + P, e * 512 : (e + 1) * 512], osb[:])


def _chunked(m):
    """[rows, cols] -> [P, rows//P, cols] with row r at [r % P, r // P]."""
    r, c = m.shape
    return np.ascontiguousarray(m.reshape(r // P, P, c).transpose(1, 0, 2))


def _pack_spec(re, im):
    """re[1025], im[1025] -> packed [2048]: re[0..1024] then im[1..1023]."""
    return np.concatenate([re, im[1:1024]])


def _constants():
    if "consts" in _CACHE:
        return _CACHE["consts"]
    d = np.arange(D, dtype=np.float64)
    f = np.arange(D // 2 + 1, dtype=np.float64)
    ang = 2.0 * np.pi / D * np.outer(d, f)  # [D, 1025]
    cos, sin = np.cos(ang), np.sin(ang)
    alpha = np.full(1025, 2.0)
    alpha[0] = alpha[1024] = 1.0
    Gf = np.concatenate(
        [(alpha[:, None] * cos.T) / D, (-2.0 * sin[:, 1:1024].T) / D], axis=0
    )  # [D packed, D]

    # packed 1024-point DFT matrix [1024 rows m, 1024 packed cols]
    m1 = np.arange(1024, dtype=np.float64)
    q1 = np.arange(513, dtype=np.float64)
    ang1 = 2.0 * np.pi / 1024 * np.outer(m1, q1)
    CS1024 = np.concatenate(
        [np.cos(ang1), -np.sin(ang1)[:, 1:512]], axis=1
    )  # [1024, 1024]

    # mirror stationaries
    Rrev = np.zeros((P, P))
    for q in range(1, P):
        Rrev[q, P - q] = 1.0
    R00 = np.zeros((P, P))
    R00[0, 0] = 1.0
    RR = np.stack([Rrev, R00, -Rrev, -R00])  # [4, P, P] (lhsT: [K, M] per slot)

    # combine scalars: CA/CB [P, 16]
    p = np.arange(P, dtype=np.float64)
    CAm = np.zeros((P, NPF))
    CBm = np.zeros((P, NPF))
    for c in range(8):  # Re side
        k = 128 * c + p
        CAm[:, c] = np.cos(2 * np.pi * k / D)
        CBm[:, c] = np.sin(2 * np.pi * k / D) * (1.0 if c <= 3 else -1.0)
    for cc in range(8, 16):  # Im side
        cp = cc - 8
        k = 128 * cp + p
        CAm[:, cc] = np.cos(2 * np.pi * k / D) * (1.0 if cp <= 3 else -1.0)
        CBm[:, cc] = -np.sin(2 * np.pi * k / D)

    consts = {
        "CS": _chunked(CS1024.astype(np.float32)).astype(bf16),  # [P, 8, 1024]
        "RR": np.ascontiguousarray(RR.transpose(1, 0, 2)).astype(bf16),  # [P,4,P]
        "CA": CAm.astype(np.float32),
        "CB": CBm.astype(np.float32),
        "Gf": Gf,
    }
    _CACHE["consts"] = consts
    return consts


def prepare_in_maps(x, queries, keyvalues, w_out):
    x = np.asarray(x, dtype=np.float32)
    queries = np.asarray(queries, dtype=np.float32)
    keyvalues = np.asarray(keyvalues, dtype=np.float32)
    w_out = np.asarray(w_out, dtype=np.float32)
    consts = _constants()

    c = (queries * keyvalues).reshape(-1)  # [1025]
    c_packed = _pack_spec(c, c)  # [2048]
    GWf = (c_packed[:, None] * consts["Gf"]).astype(np.float32) @ np.ascontiguousarray(
        w_out.T
    )  # [D packed, D out]
    GWc = _chunked(GWf).astype(bf16)

    in_maps = []
    for b in range(NB):
        for h in range(2):
            xs = x[b, h * T : (h + 1) * T]  # [T, D]
            xe = np.ascontiguousarray(xs[:, 0::2].T)  # [1024, T]
            xo = np.ascontiguousarray(xs[:, 1::2].T)
            xec = _chunked(xe)  # [P, 8, T]
            xoc = _chunked(xo)
            xcat = np.concatenate([xec, xoc], axis=1)  # [P, 16, T]
            xSc = np.ascontiguousarray(
                xcat.reshape(P, ND, NSL, TS).transpose(2, 0, 1, 3)
            ).astype(bf16)
            if h == 0:
                c0 = np.zeros((P, NPF), np.float32)
            else:
                F = np.fft.rfft(x[b, :T].sum(axis=0).astype(np.float64))
                c0 = _chunked(
                    _pack_spec(F.real, F.imag).astype(np.float32)[:, None]
                )[:, :, 0]
            in_maps.append(
                {
                    "xS": xSc,
                    "CS": consts["CS"],
                    "RR": consts["RR"],
                    "CA": consts["CA"],
                    "CB": consts["CB"],
                    "GW": GWc,
                    "C0": np.ascontiguousarray(c0),
                }
            )
    return in_maps


def kernel(x, queries, keyvalues, w_out):
    if "nc" not in _CACHE:
        _CACHE["nc"] = _build_nc()
    nc = _CACHE["nc"]
    in_maps = prepare_in_maps(x, queries, keyvalues, w_out)
    res = run_bass_kernel_spmd(nc, in_maps, core_ids=list(range(8)))
    y = np.empty((NB, NS, D), np.float32)
    for i in range(8):
        b, h = i // 2, i % 2
        y[b, h * T : (h + 1) * T] = res.results[i]["out"]
    return y


# revision 15
# speedup vs baseline: 1.3565x; 1.0001x over previous
"""HRR binding self-attention kernel for 8 trn2 NeuronCores.

Math: out = irfft(c * rfft(x) * cumsum_s(rfft(x))) @ w_out.T  with c = queries*keyvalues.
rfft is linear so cumsum commutes with it; the prefix sum runs in the frequency
domain.  irfft and the output Linear fuse into one matmul: out = qv^T @ GW with
GW = (c*G) @ w_out.T precomputed on host.

The forward rfft is radix-2 split: X[k] = E[k] + W^k O[k] with E,O = packed
rfft_1024 of even/odd samples — two 1024-wide DFT matmuls (half the PE work of
a direct 2048 DFT).  The upper half spectrum (k>512) needs partition-mirrored
E/O rows: cheap permutation matmuls (reversal + p0-pick stationaries) provide
them; conjugation signs and the W^k twiddle fold into per-partition scalars of
scalar_tensor_tensor combine ops on the DVE.

Packed spectra (1024 rows for E/O, 2048 for X): Re[0..N/2] then Im[1..N/2-1];
chunked [P=128 x chunks], chunk j pairs with chunk j+nchunks/2 on equal
partitions for complex ops; DC/Nyquist ride partition 0 with fixups.

Sharding: 8 shards = (batch b in 0..3) x (seq half h in 0..1), 2048 tokens
each; h=1 shards get the first half's spectrum sum as initial cumsum carry
(host rfft of x[b,:2048].sum(0)).

Per-core pipeline over 512-token slabs (all matmuls bf16, fp32 PSUM):
  EO-DFT -> ACT drains to SBUF -> mirror matmuls -> DVE combine -> DVE
  tensor_tensor_scan (cumsum, carry chained across slabs) -> DVE complex
  multiply -> qv bf16 -> GW matmul (one slab behind, keeps PE dense) ->
  ACT drain -> DMA out.
"""

import sys

sys.path.insert(0, "/opt/trn_rl_repo")

import numpy as np
import ml_dtypes

import concourse.bass as bass
import concourse.bacc as bacc
import concourse.mybir as mybir
from concourse.tile import TileContext
from concourse.bass_utils import run_bass_kernel_spmd

BF16 = mybir.dt.bfloat16
F32 = mybir.dt.float32
ALU = mybir.AluOpType

P = 128
D = 2048  # model dims
T = 2048  # tokens per shard
ND = D // P  # 16 chunks of x (8 even + 8 odd)
NPF = 16  # packed-frequency chunks of X
TS = 512  # tokens per slab
NSL = T // TS  # slabs
NB = 4  # batch
NS = 4096  # full seq

bf16 = ml_dtypes.bfloat16

_CACHE = {}


def _build_nc(reps: int = 1):
    nc = bacc.Bacc("TRN2", target_bir_lowering=False, debug=False, num_devices=8)
    # xS chunks 0..7 = even samples chunked, 8..15 = odd samples chunked
    xS = nc.dram_tensor("xS", [NSL, P, ND, TS], BF16, kind="ExternalInput")
    CS = nc.dram_tensor("CS", [P, 8, 1024], BF16, kind="ExternalInput")  # 1024-DFT
    RR = nc.dram_tensor("RR", [P, 4, P], BF16, kind="ExternalInput")  # Rrev,R00,-Rrev,-R00
    CA = nc.dram_tensor("CA", [P, NPF], F32, kind="ExternalInput")
    CB = nc.dram_tensor("CB", [P, NPF], F32, kind="ExternalInput")
    GW = nc.dram_tensor("GW", [P, NPF, D], BF16, kind="ExternalInput")
    C0 = nc.dram_tensor("C0", [P, NPF], F32, kind="ExternalInput")
    out = nc.dram_tensor("out", [T, D], F32, kind="ExternalOutput")

    with TileContext(nc) as tc:
        import contextlib

        loop_ctx = tc.For_i(0, reps, 1) if reps > 1 else contextlib.nullcontext()
        with loop_ctx:
            _body(nc, tc, xS, CS, RR, CA, CB, GW, C0, out)
    nc.finalize()
    return nc


def _body(nc, tc, xS, CS, RR, CA, CB, GW, C0, out):
    with (
        tc.tile_pool(name="const", bufs=1) as cpool,
        tc.tile_pool(name="xt", bufs=2) as xpool,
        tc.tile_pool(name="eo", bufs=1) as eopool,
        tc.tile_pool(name="mir", bufs=1) as mpool,
        tc.tile_pool(name="X", bufs=2) as Xpool,
        tc.tile_pool(name="qv", bufs=2) as qvpool,
        tc.tile_pool(name="s", bufs=2) as spool,
        tc.tile_pool(name="tmp", bufs=1) as tpool,
        tc.tile_pool(name="osb", bufs=2) as opool,
        tc.tile_pool(name="psEO", bufs=2, space="PSUM") as psEO,
        tc.tile_pool(name="psM", bufs=2, space="PSUM") as psM,
        tc.tile_pool(name="psG", bufs=2, space="PSUM") as psG,
    ):
        # slab 0 input first on the ring so the first DFT starts ASAP
        xt0 = xpool.tile([P, ND, TS], BF16, tag="xt")
        nc.sync.dma_start(xt0[:], xS[0])
        cs_sb = cpool.tile([P, 8, 1024], BF16)
        nc.sync.dma_start(cs_sb[:], CS[:])
        rr_sb = cpool.tile([P, 4, P], BF16)
        nc.sync.dma_start(rr_sb[:], RR[:])
        ca_sb = cpool.tile([P, NPF], F32)
        nc.sync.dma_start(ca_sb[:], CA[:])
        cb_sb = cpool.tile([P, NPF], F32)
        nc.sync.dma_start(cb_sb[:], CB[:])
        carry = cpool.tile([P, NPF], F32)
        nc.sync.dma_start(carry[:], C0[:])
        gw_sb = cpool.tile([P, NPF, D], BF16)
        for g in range(4):
            nc.sync.dma_start(
                gw_sb[:, 4 * g : 4 * (g + 1), :], GW[:, 4 * g : 4 * (g + 1), :]
            )
        zeros = cpool.tile([P, TS], BF16)
        nc.vector.memset(zeros[:], 0.0)

        qv_prev = None
        for s in range(NSL):
            if s == 0:
                xt = xt0
            else:
                xt = xpool.tile([P, ND, TS], BF16, tag="xt")
                nc.sync.dma_start(xt[:], xS[s])

            # ---- EO-DFT: 16 packed output chunks (8 E + 8 O) ----
            eo = eopool.tile([P, NPF, TS], BF16, tag="eo")
            for oc in range(NPF):
                base = 0 if oc < 8 else 8
                col = oc % 8
                ps = psEO.tile([P, TS], F32, tag="psEO")
                for d in range(8):
                    nc.tensor.matmul(
                        ps[:],
                        cs_sb[:, d, col * P : (col + 1) * P],
                        xt[:, base + d, :],
                        start=(d == 0),
                        stop=(d == 7),
                    )
                nc.scalar.copy(eo[:, oc, :], ps[:])

            # ---- mirrors for the upper half (chunks c=4..7) ----
            # M layout: 0..3 EreM, 4..7 OreM, 8..11 OimM, 12..15 -EimM
            mir = mpool.tile([P, NPF, TS], BF16, tag="mir")
            mirror_specs = []
            for i, c in enumerate(range(4, 8)):
                mirror_specs += [
                    (i, [(0, 7 - c), (1, 8 - c)]),               # EreM: Rrev,R00 on E
                    (4 + i, [(0, 8 + 7 - c), (1, 8 + 8 - c)]),   # OreM on O
                    (8 + i, [(0, 8 + 11 - c)] + ([(1, 8 + 12 - c)] if c > 4 else [])),
                    (12 + i, [(2, 11 - c)] + ([(3, 12 - c)] if c > 4 else [])),
                ]

            def emit_mirror(mslot, terms):
                pm = psM.tile([P, TS], F32, tag="psM")
                for ti, (rsel, ech) in enumerate(terms):
                    nc.tensor.matmul(
                        pm[:],
                        rr_sb[:, rsel, :],
                        eo[:, ech, :],
                        start=(ti == 0),
                        stop=(ti == len(terms) - 1),
                    )
                nc.scalar.copy(mir[:, mslot, :], pm[:])

            # ---- GW block one slab behind, mirror matmuls woven in so their
            # drains spread over the GW phase instead of bursting ----
            if qv_prev is not None:
                _gw_block(nc, tc, gw_sb, qv_prev, out, s - 1, opool, psG,
                          mirror_specs, emit_mirror)
            else:
                for mslot, terms in mirror_specs:
                    emit_mirror(mslot, terms)

            # ---- combine + scan + complex multiply, per frequency pair ----
            qv = qvpool.tile([P, NPF, TS], BF16, tag="qv")
            for j in range(8):
                X = Xpool.tile([P, 2, TS], BF16, tag="X")
                tA = tpool.tile([P, TS], BF16, tag="tA")
                if j <= 3:
                    nc.vector.scalar_tensor_tensor(
                        tA[:], eo[:, 8 + j, :], ca_sb[:, j : j + 1], eo[:, j, :],
                        ALU.mult, ALU.add,
                    )
                    nc.vector.scalar_tensor_tensor(
                        X[:, 0, :], eo[:, 12 + j, :], cb_sb[:, j : j + 1], tA[:],
                        ALU.mult, ALU.add,
                    )
                    tB = tpool.tile([P, TS], BF16, tag="tB")
                    nc.vector.scalar_tensor_tensor(
                        tB[:], eo[:, 12 + j, :], ca_sb[:, 8 + j : 9 + j], eo[:, 4 + j, :],
                        ALU.mult, ALU.add,
                    )
                    nc.vector.scalar_tensor_tensor(
                        X[:, 1, :], eo[:, 8 + j, :], cb_sb[:, 8 + j : 9 + j], tB[:],
                        ALU.mult, ALU.add,
                    )
                else:
                    i = j - 4
                    nc.vector.scalar_tensor_tensor(
                        tA[:], mir[:, 4 + i, :], ca_sb[:, j : j + 1], mir[:, i, :],
                        ALU.mult, ALU.add,
                    )
                    nc.vector.scalar_tensor_tensor(
                        X[:, 0, :], mir[:, 8 + i, :], cb_sb[:, j : j + 1], tA[:],
                        ALU.mult, ALU.add,
                    )
                    tB = tpool.tile([P, TS], BF16, tag="tB")
                    nc.vector.scalar_tensor_tensor(
                        tB[:], mir[:, 8 + i, :], ca_sb[:, 8 + j : 9 + j], mir[:, 12 + i, :],
                        ALU.mult, ALU.add,
                    )
                    nc.vector.scalar_tensor_tensor(
                        X[:, 1, :], mir[:, 4 + i, :], cb_sb[:, 8 + j : 9 + j], tB[:],
                        ALU.mult, ALU.add,
                    )
                if j == 0:
                    # X row 1024 (chunk 8, p0) is Nyquist: Re X[1024] = E[0] - O[0]
                    nc.vector.tensor_sub(
                        X[0:1, 1, :], eo[0:1, 0, :], eo[0:1, 8, :]
                    )

                S = spool.tile([P, 2, TS], BF16, tag="S")
                nc.vector.tensor_tensor_scan(
                    S[:, 0, :], X[:, 0, :], zeros[:], carry[:, j : j + 1],
                    ALU.add, ALU.add,
                )
                nc.vector.tensor_tensor_scan(
                    S[:, 1, :], X[:, 1, :], zeros[:], carry[:, j + 8 : j + 9],
                    ALU.add, ALU.add,
                )
                nc.vector.tensor_copy(carry[:, j : j + 1], S[:, 0, TS - 1 : TS])
                nc.vector.tensor_copy(carry[:, j + 8 : j + 9], S[:, 1, TS - 1 : TS])

                t1 = tpool.tile([P, TS], F32, tag="t1")
                t2 = tpool.tile([P, TS], F32, tag="t2")
                nc.vector.tensor_mul(t1[:], X[:, 0, :], S[:, 0, :])
                nc.vector.tensor_mul(t2[:], X[:, 1, :], S[:, 1, :])
                nc.vector.tensor_sub(qv[:, j, :], t1[:], t2[:])
                t3 = tpool.tile([P, TS], F32, tag="t1")
                t4 = tpool.tile([P, TS], F32, tag="t2")
                nc.vector.tensor_mul(t3[:], X[:, 0, :], S[:, 1, :])
                nc.vector.tensor_mul(t4[:], X[:, 1, :], S[:, 0, :])
                nc.vector.tensor_add(qv[:, j + 8, :], t3[:], t4[:])
                if j == 0:
                    # DC (chunk 0 p0) and Nyquist (chunk 8 p0) are purely real
                    nc.vector.tensor_mul(qv[0:1, 0, :], X[0:1, 0, :], S[0:1, 0, :])
                    nc.vector.tensor_mul(qv[0:1, 8, :], X[0:1, 1, :], S[0:1, 1, :])

            qv_prev = qv
        _gw_block(nc, tc, gw_sb, qv_prev, out, NSL - 1, opool, psG)


def _gw_block(nc, tc, gw_sb, qv, out, s, opool, psG, mirror_specs=None, emit_mirror=None):
    """out[s*TS + tg*128 + t, e] = sum_r qv[r, tg*128+t] * GW[r, e]"""
    for tg in range(TS // P):
        for eh in range(2):
            if mirror_specs:
                it = 2 * tg + eh
                for mslot, terms in mirror_specs[2 * it : 2 * it + 2]:
                    emit_mirror(mslot, terms)
            ps = psG.tile([P, 2, 512], F32, tag="psG")
            for pf in range(NPF):
                for e2 in range(2):
                    e = 2 * eh + e2
                    nc.tensor.matmul(
                        ps[:, e2, :],
                        qv[:, pf, tg * P : (tg + 1) * P],
                        gw_sb[:, pf, e * 512 : (e + 1) * 512],
                        start=(pf == 0),
                        stop=(pf == NPF - 1),
                    )
            r0 = s * TS + tg * P
            for e2 in range(2):
                osb = opool.tile([P, 512], F32, tag="osb")
                nc.scalar.copy(osb[:], ps[:, e2, :])
                e = 2 * eh + e2
                nc.sync.dma_start(out[r0 : r0 + P, e * 512 : (e + 1) * 512], osb[:])


def _chunked(m):
    """[rows, cols] -> [P, rows//P, cols] with row r at [r % P, r // P]."""
    r, c = m.shape
    return np.ascontiguousarray(m.reshape(r // P, P, c).transpose(1, 0, 2))


def _pack_spec(re, im):
    """re[1025], im[1025] -> packed [2048]: re[0..1024] then im[1..1023]."""
    return np.concatenate([re, im[1:1024]])


def _constants():
    if "consts" in _CACHE:
        return _CACHE["consts"]
    d = np.arange(D, dtype=np.float64)
    f = np.arange(D // 2 + 1, dtype=np.float64)
    ang = 2.0 * np.pi / D * np.outer(d, f)  # [D, 1025]
    cos, sin = np.cos(ang), np.sin(ang)
    alpha = np.full(1025, 2.0)
    alpha[0] = alpha[1024] = 1.0
    Gf = np.concatenate(
        [(alpha[:, None] * cos.T) / D, (-2.0 * sin[:, 1:1024].T) / D], axis=0
    )  # [D packed, D]

    # packed 1024-point DFT matrix [1024 rows m, 1024 packed cols]
    m1 = np.arange(1024, dtype=np.float64)
    q1 = np.arange(513, dtype=np.float64)
    ang1 = 2.0 * np.pi / 1024 * np.outer(m1, q1)
    CS1024 = np.concatenate(
        [np.cos(ang1), -np.sin(ang1)[:, 1:512]], axis=1
    )  # [1024, 1024]

    # mirror stationaries
    Rrev = np.zeros((P, P))
    for q in range(1, P):
        Rrev[q, P - q] = 1.0
    R00 = np.zeros((P, P))
    R00[0, 0] = 1.0
    RR = np.stack([Rrev, R00, -Rrev, -R00])  # [4, P, P] (lhsT: [K, M] per slot)

    # combine scalars: CA/CB [P, 16]
    p = np.arange(P, dtype=np.float64)
    CAm = np.zeros((P, NPF))
    CBm = np.zeros((P, NPF))
    for c in range(8):  # Re side
        k = 128 * c + p
        CAm[:, c] = np.cos(2 * np.pi * k / D)
        CBm[:, c] = np.sin(2 * np.pi * k / D) * (1.0 if c <= 3 else -1.0)
    for cc in range(8, 16):  # Im side
        cp = cc - 8
        k = 128 * cp + p
        CAm[:, cc] = np.cos(2 * np.pi * k / D) * (1.0 if cp <= 3 else -1.0)
        CBm[:, cc] = -np.sin(2 * np.pi * k / D)

    consts = {
        "CS": _chunked(CS1024.astype(np.float32)).astype(bf16),  # [P, 8, 1024]
        "RR": np.ascontiguousarray(RR.transpose(1, 0, 2)).astype(bf16),  # [P,4,P]
        "CA": CAm.astype(np.float32),
        "CB": CBm.astype(np.float32),
        "Gf": Gf,
    }
    _CACHE["consts"] = consts
    return consts


def prepare_in_maps(x, queries, keyvalues, w_out):
    x = np.asarray(x, dtype=np.float32)
    queries = np.asarray(queries, dtype=np.float32)
    keyvalues = np.asarray(keyvalues, dtype=np.float32)
    w_out = np.asarray(w_out, dtype=np.float32)
    consts = _constants()

    c = (queries * keyvalues).reshape(-1)  # [1025]
    c_packed = _pack_spec(c, c)  # [2048]
    GWf = (c_packed[:, None] * consts["Gf"]).astype(np.float32) @ np.ascontiguousarray(
        w_out.T
    )  # [D packed, D out]
    GWc = _chunked(GWf).astype(bf16)

    in_maps = []
    for b in range(NB):
        for h in range(2):
            xs = x[b, h * T : (h + 1) * T]  # [T, D]
            xe = np.ascontiguousarray(xs[:, 0::2].T)  # [1024, T]
            xo = np.ascontiguousarray(xs[:, 1::2].T)
            xec = _chunked(xe)  # [P, 8, T]
            xoc = _chunked(xo)
            xcat = np.concatenate([xec, xoc], axis=1)  # [P, 16, T]
            xSc = np.ascontiguousarray(
                xcat.reshape(P, ND, NSL, TS).transpose(2, 0, 1, 3)
            ).astype(bf16)
            if h == 0:
                c0 = np.zeros((P, NPF), np.float32)
            else:
                F = np.fft.rfft(x[b, :T].sum(axis=0).astype(np.float64))
                c0 = _chunked(
                    _pack_spec(F.real, F.imag).astype(np.float32)[:, None]
                )[:, :, 0]
            in_maps.append(
                {
                    "xS": xSc,
                    "CS": consts["CS"],
                    "RR": consts["RR"],
                    "CA": consts["CA"],
                    "CB": consts["CB"],
                    "GW": GWc,
                    "C0": np.ascontiguousarray(c0),
                }
            )
    return in_maps


def kernel(x, queries, keyvalues, w_out):
    if "nc" not in _CACHE:
        _CACHE["nc"] = _build_nc()
    nc = _CACHE["nc"]
    in_maps = prepare_in_maps(x, queries, keyvalues, w_out)
    res = run_bass_kernel_spmd(nc, in_maps, core_ids=list(range(8)))
    y = np.empty((NB, NS, D), np.float32)
    for i in range(8):
        b, h = i // 2, i % 2
        y[b, h * T : (h + 1) * T] = res.results[i]["out"]
    return y


# revision 28
# speedup vs baseline: 1.3629x; 1.0047x over previous
"""HRR binding self-attention kernel for 8 trn2 NeuronCores.

Math: out = irfft(c * rfft(x) * cumsum_s(rfft(x))) @ w_out.T  with c = queries*keyvalues.
rfft is linear so cumsum commutes with it; the prefix sum runs in the frequency
domain.  irfft and the output Linear fuse into one matmul: out = qv^T @ GW with
GW = (c*G) @ w_out.T precomputed on host.

The forward rfft is radix-2 split: X[k] = E[k] + W^k O[k] with E,O = packed
rfft_1024 of even/odd samples — two 1024-wide DFT matmuls (half the PE work of
a direct 2048 DFT).  The upper half spectrum (k>512) needs partition-mirrored
E/O rows: cheap permutation matmuls (reversal + p0-pick stationaries) provide
them; conjugation signs and the W^k twiddle fold into per-partition scalars of
scalar_tensor_tensor combine ops on the DVE.

Packed spectra (1024 rows for E/O, 2048 for X): Re[0..N/2] then Im[1..N/2-1];
chunked [P=128 x chunks], chunk j pairs with chunk j+nchunks/2 on equal
partitions for complex ops; DC/Nyquist ride partition 0 with fixups.

Sharding: 8 shards = (batch b in 0..3) x (seq half h in 0..1), 2048 tokens
each; h=1 shards get the first half's spectrum sum as initial cumsum carry
(host rfft of x[b,:2048].sum(0)).

Per-core pipeline over 512-token slabs (all matmuls bf16, fp32 PSUM):
  EO-DFT -> ACT drains to SBUF -> mirror matmuls -> DVE combine -> DVE
  tensor_tensor_scan (cumsum, carry chained across slabs) -> DVE complex
  multiply -> qv bf16 -> GW matmul (one slab behind, keeps PE dense) ->
  ACT drain -> DMA out.
"""

import sys

sys.path.insert(0, "/opt/trn_rl_repo")

import numpy as np
import ml_dtypes

import concourse.bass as bass
import concourse.bacc as bacc
import concourse.mybir as mybir
from concourse.tile import TileContext
from concourse.bass_utils import run_bass_kernel_spmd

BF16 = mybir.dt.bfloat16
F32 = mybir.dt.float32
ALU = mybir.AluOpType

P = 128
D = 2048  # model dims
T = 2048  # tokens per shard
ND = D // P  # 16 chunks of x (8 even + 8 odd)
NPF = 16  # packed-frequency chunks of X
TS = 512  # tokens per slab
NSL = T // TS  # slabs
NB = 4  # batch
NS = 4096  # full seq

bf16 = ml_dtypes.bfloat16

_CACHE = {}


def _build_nc(reps: int = 1):
    nc = bacc.Bacc("TRN2", target_bir_lowering=False, debug=False, num_devices=8)
    # xS chunk 4s+q = sequence s (s00=x[0::4], s01=x[2::4], s10=x[1::4],
    # s11=x[3::4]) chunked by 128 rows
    xS = nc.dram_tensor("xS", [NSL, P, ND, TS], BF16, kind="ExternalInput")
    CS = nc.dram_tensor("CS", [P, 4, 512], BF16, kind="ExternalInput")  # 512-DFT
    RR = nc.dram_tensor("RR", [P, 4, P], BF16, kind="ExternalInput")  # Rrev,R00,-Rrev,-R00
    CA = nc.dram_tensor("CA", [P, NPF], F32, kind="ExternalInput")
    CB = nc.dram_tensor("CB", [P, NPF], F32, kind="ExternalInput")
    CA1 = nc.dram_tensor("CA1", [P, 8], F32, kind="ExternalInput")
    CB1 = nc.dram_tensor("CB1", [P, 8], F32, kind="ExternalInput")
    GW = nc.dram_tensor("GW", [P, NPF, D], BF16, kind="ExternalInput")
    C0 = nc.dram_tensor("C0", [P, NPF], F32, kind="ExternalInput")
    out = nc.dram_tensor("out", [T, D], F32, kind="ExternalOutput")

    with TileContext(nc) as tc:
        import contextlib

        loop_ctx = tc.For_i(0, reps, 1) if reps > 1 else contextlib.nullcontext()
        with loop_ctx:
            _body(nc, tc, xS, CS, RR, CA, CB, CA1, CB1, GW, C0, out)
    nc.finalize()
    return nc


def _body(nc, tc, xS, CS, RR, CA, CB, CA1, CB1, GW, C0, out):
    with (
        tc.tile_pool(name="const", bufs=1) as cpool,
        tc.tile_pool(name="xt", bufs=2) as xpool,
        tc.tile_pool(name="eeoo_mir", bufs=1) as qpool,
        tc.tile_pool(name="mirL1", bufs=1) as m1pool,
        tc.tile_pool(name="eo", bufs=1) as eopool,
        tc.tile_pool(name="X", bufs=1) as Xpool,
        tc.tile_pool(name="qv", bufs=2) as qvpool,
        tc.tile_pool(name="s", bufs=1) as spool,
        tc.tile_pool(name="tmp", bufs=1) as tpool,
        tc.tile_pool(name="gt", bufs=1) as gtpool,
        tc.tile_pool(name="osb", bufs=2) as opool,
        tc.tile_pool(name="psEO", bufs=2, space="PSUM") as psEO,
        tc.tile_pool(name="psM", bufs=2, space="PSUM") as psM,
        tc.tile_pool(name="psG", bufs=2, space="PSUM") as psG,
    ):
        # slab 0 input first on the ring so the first DFT starts ASAP
        xt0 = xpool.tile([P, ND, TS], BF16, tag="xt")
        nc.sync.dma_start(xt0[:], xS[0])
        cs_sb = cpool.tile([P, 4, 512], BF16)
        nc.sync.dma_start(cs_sb[:], CS[:])
        rr_sb = cpool.tile([P, 4, P], BF16)
        nc.sync.dma_start(rr_sb[:], RR[:])
        ca_sb = cpool.tile([P, NPF], F32)
        nc.sync.dma_start(ca_sb[:], CA[:])
        cb_sb = cpool.tile([P, NPF], F32)
        nc.sync.dma_start(cb_sb[:], CB[:])
        ca1_sb = cpool.tile([P, 8], F32)
        nc.sync.dma_start(ca1_sb[:], CA1[:])
        cb1_sb = cpool.tile([P, 8], F32)
        nc.sync.dma_start(cb1_sb[:], CB1[:])
        carry = cpool.tile([P, NPF], F32)
        nc.sync.dma_start(carry[:], C0[:])
        gw_sb = cpool.tile([P, NPF, D], BF16)
        for g in range(4):
            nc.sync.dma_start(
                gw_sb[:, 4 * g : 4 * (g + 1), :], GW[:, 4 * g : 4 * (g + 1), :]
            )
        zeros = cpool.tile([P, TS], BF16)
        nc.vector.memset(zeros[:], 0.0)

        qv_prev = None
        for s in range(NSL):
            if s == 0:
                xt = xt0
            else:
                xt = xpool.tile([P, ND, TS], BF16, tag="xt")
                nc.sync.dma_start(xt[:], xS[s])

            # ---- 512-DFT: 16 packed output chunks (4 per sequence) ----
            eeoo = qpool.tile([P, NPF, TS], BF16, tag="em")
            for oc in range(NPF):
                base = 4 * (oc // 4)
                col = oc % 4
                ps = psEO.tile([P, TS], F32, tag="psEO")
                for d in range(4):
                    nc.tensor.matmul(
                        ps[:],
                        cs_sb[:, d, col * P : (col + 1) * P],
                        xt[:, base + d, :],
                        start=(d == 0),
                        stop=(d == 3),
                    )
                nc.scalar.copy(eeoo[:, oc, :], ps[:])

            # ---- L1: two 512->1024 combines (E from s00,s01; O from s10,s11) ----
            # mirror slots per pair pr at 8*pr: 0,1 AreM[c=2,3]; 2,3 BreM;
            # 4,5 BimM; 6,7 -AimM
            mir1 = m1pool.tile([P, NPF, TS], BF16, tag="mir1")
            for pr in range(2):
                a0, b0 = 8 * pr, 8 * pr + 4
                mb = 8 * pr
                l1specs = [
                    (mb + 0, [(0, a0 + 1), (1, a0 + 2)]),
                    (mb + 1, [(0, a0 + 0), (1, a0 + 1)]),
                    (mb + 2, [(0, b0 + 1), (1, b0 + 2)]),
                    (mb + 3, [(0, b0 + 0), (1, b0 + 1)]),
                    (mb + 4, [(0, b0 + 3)]),
                    (mb + 5, [(0, b0 + 2), (1, b0 + 3)]),
                    (mb + 6, [(2, a0 + 3)]),
                    (mb + 7, [(2, a0 + 2), (3, a0 + 3)]),
                ]
                for mslot, terms in l1specs:
                    pm = psM.tile([P, TS], F32, tag="psM")
                    for ti, (rsel, ech) in enumerate(terms):
                        nc.tensor.matmul(
                            pm[:],
                            rr_sb[:, rsel, :],
                            eeoo[:, ech, :],
                            start=(ti == 0),
                            stop=(ti == len(terms) - 1),
                        )
                    nc.scalar.copy(mir1[:, mslot, :], pm[:])

            # L1 combines on GPSIMD (PE and DVE stay free)
            eo = eopool.tile([P, NPF, TS], BF16, tag="eo")
            for pr in range(2):
                eng = nc.vector  # TensorScalarPtr is not legal on Pool/GPSIMD
                a0, b0, e0, mb = 8 * pr, 8 * pr + 4, 8 * pr, 8 * pr
                for c in range(4):  # Re side of the 1024-spectrum
                    gt = gtpool.tile([P, TS], BF16, tag=f"gt{pr}")
                    if c < 2:
                        eng.scalar_tensor_tensor(
                            gt[:], eeoo[:, b0 + c, :], ca1_sb[:, c : c + 1],
                            eeoo[:, a0 + c, :], ALU.mult, ALU.add,
                        )
                        eng.scalar_tensor_tensor(
                            eo[:, e0 + c, :], eeoo[:, b0 + 2 + c, :],
                            cb1_sb[:, c : c + 1], gt[:], ALU.mult, ALU.add,
                        )
                    else:
                        i = c - 2
                        eng.scalar_tensor_tensor(
                            gt[:], mir1[:, mb + 2 + i, :], ca1_sb[:, c : c + 1],
                            mir1[:, mb + i, :], ALU.mult, ALU.add,
                        )
                        eng.scalar_tensor_tensor(
                            eo[:, e0 + c, :], mir1[:, mb + 4 + i, :],
                            cb1_sb[:, c : c + 1], gt[:], ALU.mult, ALU.add,
                        )
                for cc in range(4):  # Im side
                    gt = gtpool.tile([P, TS], BF16, tag=f"gt{pr}")
                    if cc < 2:
                        eng.scalar_tensor_tensor(
                            gt[:], eeoo[:, b0 + 2 + cc, :], ca1_sb[:, 4 + cc : 5 + cc],
                            eeoo[:, a0 + 2 + cc, :], ALU.mult, ALU.add,
                        )
                        eng.scalar_tensor_tensor(
                            eo[:, e0 + 4 + cc, :], eeoo[:, b0 + cc, :],
                            cb1_sb[:, 4 + cc : 5 + cc], gt[:], ALU.mult, ALU.add,
                        )
                    else:
                        i = cc - 2
                        eng.scalar_tensor_tensor(
                            gt[:], mir1[:, mb + 4 + i, :], ca1_sb[:, 4 + cc : 5 + cc],
                            mir1[:, mb + 6 + i, :], ALU.mult, ALU.add,
                        )
                        eng.scalar_tensor_tensor(
                            eo[:, e0 + 4 + cc, :], mir1[:, mb + 2 + i, :],
                            cb1_sb[:, 4 + cc : 5 + cc], gt[:], ALU.mult, ALU.add,
                        )
                # 1024-Nyquist row (chunk e0+4, p0): A[0] - B[0]
                eng.tensor_sub(
                    eo[0:1, e0 + 4, :], eeoo[0:1, a0, :], eeoo[0:1, b0, :]
                )

            # ---- mirrors for the upper half (chunks c=4..7) ----
            # M layout: 0..3 EreM, 4..7 OreM, 8..11 OimM, 12..15 -EimM
            mir = qpool.tile([P, NPF, TS], BF16, tag="em")
            mirror_specs = []
            for i, c in enumerate(range(4, 8)):
                mirror_specs += [
                    (i, [(0, 7 - c), (1, 8 - c)]),               # EreM: Rrev,R00 on E
                    (4 + i, [(0, 8 + 7 - c), (1, 8 + 8 - c)]),   # OreM on O
                    (8 + i, [(0, 8 + 11 - c)] + ([(1, 8 + 12 - c)] if c > 4 else [])),
                    (12 + i, [(2, 11 - c)] + ([(3, 12 - c)] if c > 4 else [])),
                ]

            def emit_mirror(mslot, terms):
                pm = psM.tile([P, TS], F32, tag="psM")
                for ti, (rsel, ech) in enumerate(terms):
                    nc.tensor.matmul(
                        pm[:],
                        rr_sb[:, rsel, :],
                        eo[:, ech, :],
                        start=(ti == 0),
                        stop=(ti == len(terms) - 1),
                    )
                nc.scalar.copy(mir[:, mslot, :], pm[:])

            # ---- GW block one slab behind, mirror matmuls woven in so their
            # drains spread over the GW phase instead of bursting ----
            if qv_prev is not None:
                _gw_block(nc, tc, gw_sb, qv_prev, out, s - 1, opool, psG,
                          mirror_specs, emit_mirror)
            else:
                for mslot, terms in mirror_specs:
                    emit_mirror(mslot, terms)

            # ---- combine + scan + complex multiply, per frequency pair ----
            qv = qvpool.tile([P, NPF, TS], BF16, tag="qv")
            for j in range(8):
                X = Xpool.tile([P, 2, TS], BF16, tag="X")
                tA = tpool.tile([P, TS], BF16, tag="tA")
                if j <= 3:
                    nc.vector.scalar_tensor_tensor(
                        tA[:], eo[:, 8 + j, :], ca_sb[:, j : j + 1], eo[:, j, :],
                        ALU.mult, ALU.add,
                    )
                    nc.vector.scalar_tensor_tensor(
                        X[:, 0, :], eo[:, 12 + j, :], cb_sb[:, j : j + 1], tA[:],
                        ALU.mult, ALU.add,
                    )
                    tB = tpool.tile([P, TS], BF16, tag="tB")
                    nc.vector.scalar_tensor_tensor(
                        tB[:], eo[:, 12 + j, :], ca_sb[:, 8 + j : 9 + j], eo[:, 4 + j, :],
                        ALU.mult, ALU.add,
                    )
                    nc.vector.scalar_tensor_tensor(
                        X[:, 1, :], eo[:, 8 + j, :], cb_sb[:, 8 + j : 9 + j], tB[:],
                        ALU.mult, ALU.add,
                    )
                else:
                    i = j - 4
                    nc.vector.scalar_tensor_tensor(
                        tA[:], mir[:, 4 + i, :], ca_sb[:, j : j + 1], mir[:, i, :],
                        ALU.mult, ALU.add,
                    )
                    nc.vector.scalar_tensor_tensor(
                        X[:, 0, :], mir[:, 8 + i, :], cb_sb[:, j : j + 1], tA[:],
                        ALU.mult, ALU.add,
                    )
                    tB = tpool.tile([P, TS], BF16, tag="tB")
                    nc.vector.scalar_tensor_tensor(
                        tB[:], mir[:, 8 + i, :], ca_sb[:, 8 + j : 9 + j], mir[:, 12 + i, :],
                        ALU.mult, ALU.add,
                    )
                    nc.vector.scalar_tensor_tensor(
                        X[:, 1, :], mir[:, 4 + i, :], cb_sb[:, 8 + j : 9 + j], tB[:],
                        ALU.mult, ALU.add,
                    )
                if j == 0:
                    # X row 1024 (chunk 8, p0) is Nyquist: Re X[1024] = E[0] - O[0]
                    nc.vector.tensor_sub(
                        X[0:1, 1, :], eo[0:1, 0, :], eo[0:1, 8, :]
                    )

                S = spool.tile([P, 2, TS], BF16, tag="S")
                nc.vector.tensor_tensor_scan(
                    S[:, 0, :], X[:, 0, :], zeros[:], carry[:, j : j + 1],
                    ALU.add, ALU.add,
                )
                nc.vector.tensor_tensor_scan(
                    S[:, 1, :], X[:, 1, :], zeros[:], carry[:, j + 8 : j + 9],
                    ALU.add, ALU.add,
                )
                nc.vector.tensor_copy(carry[:, j : j + 1], S[:, 0, TS - 1 : TS])
                nc.vector.tensor_copy(carry[:, j + 8 : j + 9], S[:, 1, TS - 1 : TS])

                t1 = tpool.tile([P, TS], BF16, tag="t1")
                t2 = tpool.tile([P, TS], BF16, tag="t2")
                nc.vector.tensor_mul(t1[:], X[:, 0, :], S[:, 0, :])
                nc.vector.tensor_mul(t2[:], X[:, 1, :], S[:, 1, :])
                nc.vector.tensor_sub(qv[:, j, :], t1[:], t2[:])
                t3 = tpool.tile([P, TS], BF16, tag="t1")
                t4 = tpool.tile([P, TS], BF16, tag="t2")
                nc.vector.tensor_mul(t3[:], X[:, 0, :], S[:, 1, :])
                nc.vector.tensor_mul(t4[:], X[:, 1, :], S[:, 0, :])
                nc.vector.tensor_add(qv[:, j + 8, :], t3[:], t4[:])
                if j == 0:
                    # DC (chunk 0 p0) and Nyquist (chunk 8 p0) are purely real
                    nc.vector.tensor_mul(qv[0:1, 0, :], X[0:1, 0, :], S[0:1, 0, :])
                    nc.vector.tensor_mul(qv[0:1, 8, :], X[0:1, 1, :], S[0:1, 1, :])

            qv_prev = qv
        _gw_block(nc, tc, gw_sb, qv_prev, out, NSL - 1, opool, psG)


def _gw_block(nc, tc, gw_sb, qv, out, s, opool, psG, mirror_specs=None, emit_mirror=None):
    """out[s*TS + tg*128 + t, e] = sum_r qv[r, tg*128+t] * GW[r, e]"""
    for tg in range(TS // P):
        for eh in range(2):
            if mirror_specs:
                # L0 mirrors depend on the gpsimd L1-combine chain; weave them
                # into the later GW iterations so the PE never waits on it
                it = 2 * tg + eh
                if it in (5, 6):
                    for mslot, terms in mirror_specs[8 * (it - 5) : 8 * (it - 5) + 8]:
                        emit_mirror(mslot, terms)
            ps = psG.tile([P, 2, 512], F32, tag="psG")
            for pf in range(NPF):
                for e2 in range(2):
                    e = 2 * eh + e2
                    nc.tensor.matmul(
                        ps[:, e2, :],
                        qv[:, pf, tg * P : (tg + 1) * P],
                        gw_sb[:, pf, e * 512 : (e + 1) * 512],
                        start=(pf == 0),
                        stop=(pf == NPF - 1),
                    )
            r0 = s * TS + tg * P
            for e2 in range(2):
                osb = opool.tile([P, 512], F32, tag="osb")
                nc.scalar.copy(osb[:], ps[:, e2, :])
                e = 2 * eh + e2
                nc.sync.dma_start(out[r0 : r0 + P, e * 512 : (e + 1) * 512], osb[:])


def _chunked(m):
    """[rows, cols] -> [P, rows//P, cols] with row r at [r % P, r // P]."""
    r, c = m.shape
    return np.ascontiguousarray(m.reshape(r // P, P, c).transpose(1, 0, 2))


def _pack_spec(re, im):
    """re[1025], im[1025] -> packed [2048]: re[0..1024] then im[1..1023]."""
    return np.concatenate([re, im[1:1024]])


def _constants():
    if "consts" in _CACHE:
        return _CACHE["consts"]
    d = np.arange(D, dtype=np.float64)
    f = np.arange(D // 2 + 1, dtype=np.float64)
    ang = 2.0 * np.pi / D * np.outer(d, f)  # [D, 1025]
    cos, sin = np.cos(ang), np.sin(ang)
    alpha = np.full(1025, 2.0)
    alpha[0] = alpha[1024] = 1.0
    Gf = np.concatenate(
        [(alpha[:, None] * cos.T) / D, (-2.0 * sin[:, 1:1024].T) / D], axis=0
    )  # [D packed, D]

    # packed 512-point DFT matrix [512 rows m, 512 packed cols]
    m1 = np.arange(512, dtype=np.float64)
    q1 = np.arange(257, dtype=np.float64)
    ang1 = 2.0 * np.pi / 512 * np.outer(m1, q1)
    CS512 = np.concatenate(
        [np.cos(ang1), -np.sin(ang1)[:, 1:256]], axis=1
    )  # [512, 512]

    # mirror stationaries
    Rrev = np.zeros((P, P))
    for q in range(1, P):
        Rrev[q, P - q] = 1.0
    R00 = np.zeros((P, P))
    R00[0, 0] = 1.0
    RR = np.stack([Rrev, R00, -Rrev, -R00])  # [4, P, P] (lhsT: [K, M] per slot)

    # combine scalars for a radix-2 level producing a 2N-point spectrum with
    # 2h Re-chunks: CA/CB [P, 4h]
    p = np.arange(P, dtype=np.float64)

    def tables(h, twoN):
        CAm = np.zeros((P, 4 * h))
        CBm = np.zeros((P, 4 * h))
        for c in range(2 * h):  # Re side
            k = 128 * c + p
            CAm[:, c] = np.cos(2 * np.pi * k / twoN)
            CBm[:, c] = np.sin(2 * np.pi * k / twoN) * (1.0 if c < h else -1.0)
        for cc in range(2 * h):  # Im side
            k = 128 * cc + p
            CAm[:, 2 * h + cc] = np.cos(2 * np.pi * k / twoN) * (
                1.0 if cc < h else -1.0
            )
            CBm[:, 2 * h + cc] = -np.sin(2 * np.pi * k / twoN)
        return CAm.astype(np.float32), CBm.astype(np.float32)

    CAm, CBm = tables(4, D)
    CA1m, CB1m = tables(2, 1024)

    consts = {
        "CS": _chunked(CS512.astype(np.float32)).astype(bf16),  # [P, 4, 512]
        "RR": np.ascontiguousarray(RR.transpose(1, 0, 2)).astype(bf16),  # [P,4,P]
        "CA": CAm,
        "CB": CBm,
        "CA1": CA1m,
        "CB1": CB1m,
        "Gf": Gf,
    }
    _CACHE["consts"] = consts
    return consts


def prepare_in_maps(x, queries, keyvalues, w_out):
    x = np.asarray(x, dtype=np.float32)
    queries = np.asarray(queries, dtype=np.float32)
    keyvalues = np.asarray(keyvalues, dtype=np.float32)
    w_out = np.asarray(w_out, dtype=np.float32)
    consts = _constants()

    c = (queries * keyvalues).reshape(-1)  # [1025]
    c_packed = _pack_spec(c, c)  # [2048]
    GWf = (c_packed[:, None] * consts["Gf"]).astype(np.float32) @ np.ascontiguousarray(
        w_out.T
    )  # [D packed, D out]
    GWc = _chunked(GWf).astype(bf16)

    in_maps = []
    for b in range(NB):
        for h in range(2):
            xs = x[b, h * T : (h + 1) * T]  # [T, D]
            xcat = np.concatenate(
                [
                    _chunked(np.ascontiguousarray(xs[:, off::4].T))  # [P, 4, T]
                    for off in (0, 2, 1, 3)  # s00, s01, s10, s11
                ],
                axis=1,
            )  # [P, 16, T]
            xSc = np.ascontiguousarray(
                xcat.reshape(P, ND, NSL, TS).transpose(2, 0, 1, 3)
            ).astype(bf16)
            if h == 0:
                c0 = np.zeros((P, NPF), np.float32)
            else:
                F = np.fft.rfft(x[b, :T].sum(axis=0).astype(np.float64))
                c0 = _chunked(
                    _pack_spec(F.real, F.imag).astype(np.float32)[:, None]
                )[:, :, 0]
            in_maps.append(
                {
                    "xS": xSc,
                    "CS": consts["CS"],
                    "RR": consts["RR"],
                    "CA": consts["CA"],
                    "CB": consts["CB"],
                    "CA1": consts["CA1"],
                    "CB1": consts["CB1"],
                    "GW": GWc,
                    "C0": np.ascontiguousarray(c0),
                }
            )
    return in_maps


def kernel(x, queries, keyvalues, w_out):
    if "nc" not in _CACHE:
        _CACHE["nc"] = _build_nc()
    nc = _CACHE["nc"]
    in_maps = prepare_in_maps(x, queries, keyvalues, w_out)
    res = run_bass_kernel_spmd(nc, in_maps, core_ids=list(range(8)))
    y = np.empty((NB, NS, D), np.float32)
    for i in range(8):
        b, h = i // 2, i % 2
        y[b, h * T : (h + 1) * T] = res.results[i]["out"]
    return y


# revision 29
# speedup vs baseline: 1.3693x; 1.0047x over previous
"""HRR binding self-attention kernel for 8 trn2 NeuronCores.

Math: out = irfft(c * rfft(x) * cumsum_s(rfft(x))) @ w_out.T  with c = queries*keyvalues.
rfft is linear so cumsum commutes with it; the prefix sum runs in the frequency
domain.  irfft and the output Linear fuse into one matmul: out = qv^T @ GW with
GW = (c*G) @ w_out.T precomputed on host.

The forward rfft is radix-2 split: X[k] = E[k] + W^k O[k] with E,O = packed
rfft_1024 of even/odd samples — two 1024-wide DFT matmuls (half the PE work of
a direct 2048 DFT).  The upper half spectrum (k>512) needs partition-mirrored
E/O rows: cheap permutation matmuls (reversal + p0-pick stationaries) provide
them; conjugation signs and the W^k twiddle fold into per-partition scalars of
scalar_tensor_tensor combine ops on the DVE.

Packed spectra (1024 rows for E/O, 2048 for X): Re[0..N/2] then Im[1..N/2-1];
chunked [P=128 x chunks], chunk j pairs with chunk j+nchunks/2 on equal
partitions for complex ops; DC/Nyquist ride partition 0 with fixups.

Sharding: 8 shards = (batch b in 0..3) x (seq half h in 0..1), 2048 tokens
each; h=1 shards get the first half's spectrum sum as initial cumsum carry
(host rfft of x[b,:2048].sum(0)).

Per-core pipeline over 512-token slabs (all matmuls bf16, fp32 PSUM):
  EO-DFT -> ACT drains to SBUF -> mirror matmuls -> DVE combine -> DVE
  tensor_tensor_scan (cumsum, carry chained across slabs) -> DVE complex
  multiply -> qv bf16 -> GW matmul (one slab behind, keeps PE dense) ->
  ACT drain -> DMA out.
"""

import sys

sys.path.insert(0, "/opt/trn_rl_repo")

import numpy as np
import ml_dtypes

import concourse.bass as bass
import concourse.bacc as bacc
import concourse.mybir as mybir
from concourse.tile import TileContext
from concourse.bass_utils import run_bass_kernel_spmd

BF16 = mybir.dt.bfloat16
F32 = mybir.dt.float32
ALU = mybir.AluOpType

P = 128
D = 2048  # model dims
T = 2048  # tokens per shard
ND = D // P  # 16 chunks of x (8 even + 8 odd)
NPF = 16  # packed-frequency chunks of X
TS = 512  # tokens per slab
NSL = T // TS  # slabs
NB = 4  # batch
NS = 4096  # full seq

bf16 = ml_dtypes.bfloat16

_CACHE = {}


def _build_nc(reps: int = 1):
    nc = bacc.Bacc("TRN2", target_bir_lowering=False, debug=False, num_devices=8)
    # xS chunk 4s+q = sequence s (s00=x[0::4], s01=x[2::4], s10=x[1::4],
    # s11=x[3::4]) chunked by 128 rows
    xS = nc.dram_tensor("xS", [NSL, P, ND, TS], BF16, kind="ExternalInput")
    CS = nc.dram_tensor("CS", [P, 4, 512], BF16, kind="ExternalInput")  # 512-DFT
    RR = nc.dram_tensor("RR", [P, 4, P], BF16, kind="ExternalInput")  # Rrev,R00,-Rrev,-R00
    CA = nc.dram_tensor("CA", [P, NPF], F32, kind="ExternalInput")
    CB = nc.dram_tensor("CB", [P, NPF], F32, kind="ExternalInput")
    CA1 = nc.dram_tensor("CA1", [P, 8], F32, kind="ExternalInput")
    CB1 = nc.dram_tensor("CB1", [P, 8], F32, kind="ExternalInput")
    GW = nc.dram_tensor("GW", [P, NPF, D], BF16, kind="ExternalInput")
    C0 = nc.dram_tensor("C0", [P, NPF], F32, kind="ExternalInput")
    out = nc.dram_tensor("out", [T, D], F32, kind="ExternalOutput")

    with TileContext(nc) as tc:
        import contextlib

        loop_ctx = tc.For_i(0, reps, 1) if reps > 1 else contextlib.nullcontext()
        with loop_ctx:
            _body(nc, tc, xS, CS, RR, CA, CB, CA1, CB1, GW, C0, out)
    nc.finalize()
    return nc


def _body(nc, tc, xS, CS, RR, CA, CB, CA1, CB1, GW, C0, out):
    with (
        tc.tile_pool(name="const", bufs=1) as cpool,
        tc.tile_pool(name="xt", bufs=2) as xpool,
        tc.tile_pool(name="eeoo_mir", bufs=1) as qpool,
        tc.tile_pool(name="mirL1", bufs=1) as m1pool,
        tc.tile_pool(name="eo", bufs=1) as eopool,
        tc.tile_pool(name="X", bufs=1) as Xpool,
        tc.tile_pool(name="qv", bufs=2) as qvpool,
        tc.tile_pool(name="s", bufs=1) as spool,
        tc.tile_pool(name="tmp", bufs=1) as tpool,
        tc.tile_pool(name="gt", bufs=1) as gtpool,
        tc.tile_pool(name="osb", bufs=2) as opool,
        tc.tile_pool(name="psEO", bufs=2, space="PSUM") as psEO,
        tc.tile_pool(name="psM", bufs=2, space="PSUM") as psM,
        tc.tile_pool(name="psG", bufs=2, space="PSUM") as psG,
    ):
        # slab 0 input first on the ring so the first DFT starts ASAP
        xt0 = xpool.tile([P, ND, TS], BF16, tag="xt")
        nc.sync.dma_start(xt0[:], xS[0])
        cs_sb = cpool.tile([P, 4, 512], BF16)
        nc.sync.dma_start(cs_sb[:], CS[:])
        rr_sb = cpool.tile([P, 4, P], BF16)
        nc.sync.dma_start(rr_sb[:], RR[:])
        ca_sb = cpool.tile([P, NPF], F32)
        nc.sync.dma_start(ca_sb[:], CA[:])
        cb_sb = cpool.tile([P, NPF], F32)
        nc.sync.dma_start(cb_sb[:], CB[:])
        ca1_sb = cpool.tile([P, 8], F32)
        nc.sync.dma_start(ca1_sb[:], CA1[:])
        cb1_sb = cpool.tile([P, 8], F32)
        nc.sync.dma_start(cb1_sb[:], CB1[:])
        carry = cpool.tile([P, NPF], F32)
        nc.sync.dma_start(carry[:], C0[:])
        gw_sb = cpool.tile([P, NPF, D], BF16)
        for g in range(4):
            nc.sync.dma_start(
                gw_sb[:, 4 * g : 4 * (g + 1), :], GW[:, 4 * g : 4 * (g + 1), :]
            )
        zeros = cpool.tile([P, TS], BF16)
        nc.vector.memset(zeros[:], 0.0)

        qv_prev = None
        for s in range(NSL):
            if s == 0:
                xt = xt0
            else:
                xt = xpool.tile([P, ND, TS], BF16, tag="xt")
                nc.sync.dma_start(xt[:], xS[s])

            # ---- 512-DFT (16 packed chunks, 4 per sequence) with each pair's
            # L1 mirrors emitted as soon as its two sequences are drained, so
            # the DVE combine chain starts ~7us earlier ----
            eeoo = qpool.tile([P, NPF, TS], BF16, tag="em")
            mir1 = m1pool.tile([P, NPF, TS], BF16, tag="mir1")

            def emit_l1_mirrors(pr):
                a0, b0 = 8 * pr, 8 * pr + 4
                mb = 8 * pr
                l1specs = [
                    (mb + 0, [(0, a0 + 1), (1, a0 + 2)]),
                    (mb + 1, [(0, a0 + 0), (1, a0 + 1)]),
                    (mb + 2, [(0, b0 + 1), (1, b0 + 2)]),
                    (mb + 3, [(0, b0 + 0), (1, b0 + 1)]),
                    (mb + 4, [(0, b0 + 3)]),
                    (mb + 5, [(0, b0 + 2), (1, b0 + 3)]),
                    (mb + 6, [(2, a0 + 3)]),
                    (mb + 7, [(2, a0 + 2), (3, a0 + 3)]),
                ]
                for mslot, terms in l1specs:
                    pm = psM.tile([P, TS], F32, tag="psM")
                    for ti, (rsel, ech) in enumerate(terms):
                        nc.tensor.matmul(
                            pm[:],
                            rr_sb[:, rsel, :],
                            eeoo[:, ech, :],
                            start=(ti == 0),
                            stop=(ti == len(terms) - 1),
                        )
                    nc.scalar.copy(mir1[:, mslot, :], pm[:])

            for oc in range(NPF):
                base = 4 * (oc // 4)
                col = oc % 4
                ps = psEO.tile([P, TS], F32, tag="psEO")
                for d in range(4):
                    nc.tensor.matmul(
                        ps[:],
                        cs_sb[:, d, col * P : (col + 1) * P],
                        xt[:, base + d, :],
                        start=(d == 0),
                        stop=(d == 3),
                    )
                nc.scalar.copy(eeoo[:, oc, :], ps[:])
                if oc == 7:
                    emit_l1_mirrors(0)
            emit_l1_mirrors(1)

            # L1 combines on GPSIMD (PE and DVE stay free)
            eo = eopool.tile([P, NPF, TS], BF16, tag="eo")
            for pr in range(2):
                eng = nc.vector  # TensorScalarPtr is not legal on Pool/GPSIMD
                a0, b0, e0, mb = 8 * pr, 8 * pr + 4, 8 * pr, 8 * pr
                for c in range(4):  # Re side of the 1024-spectrum
                    gt = gtpool.tile([P, TS], BF16, tag=f"gt{pr}")
                    if c < 2:
                        eng.scalar_tensor_tensor(
                            gt[:], eeoo[:, b0 + c, :], ca1_sb[:, c : c + 1],
                            eeoo[:, a0 + c, :], ALU.mult, ALU.add,
                        )
                        eng.scalar_tensor_tensor(
                            eo[:, e0 + c, :], eeoo[:, b0 + 2 + c, :],
                            cb1_sb[:, c : c + 1], gt[:], ALU.mult, ALU.add,
                        )
                    else:
                        i = c - 2
                        eng.scalar_tensor_tensor(
                            gt[:], mir1[:, mb + 2 + i, :], ca1_sb[:, c : c + 1],
                            mir1[:, mb + i, :], ALU.mult, ALU.add,
                        )
                        eng.scalar_tensor_tensor(
                            eo[:, e0 + c, :], mir1[:, mb + 4 + i, :],
                            cb1_sb[:, c : c + 1], gt[:], ALU.mult, ALU.add,
                        )
                for cc in range(4):  # Im side
                    gt = gtpool.tile([P, TS], BF16, tag=f"gt{pr}")
                    if cc < 2:
                        eng.scalar_tensor_tensor(
                            gt[:], eeoo[:, b0 + 2 + cc, :], ca1_sb[:, 4 + cc : 5 + cc],
                            eeoo[:, a0 + 2 + cc, :], ALU.mult, ALU.add,
                        )
                        eng.scalar_tensor_tensor(
                            eo[:, e0 + 4 + cc, :], eeoo[:, b0 + cc, :],
                            cb1_sb[:, 4 + cc : 5 + cc], gt[:], ALU.mult, ALU.add,
                        )
                    else:
                        i = cc - 2
                        eng.scalar_tensor_tensor(
                            gt[:], mir1[:, mb + 4 + i, :], ca1_sb[:, 4 + cc : 5 + cc],
                            mir1[:, mb + 6 + i, :], ALU.mult, ALU.add,
                        )
                        eng.scalar_tensor_tensor(
                            eo[:, e0 + 4 + cc, :], mir1[:, mb + 2 + i, :],
                            cb1_sb[:, 4 + cc : 5 + cc], gt[:], ALU.mult, ALU.add,
                        )
                # 1024-Nyquist row (chunk e0+4, p0): A[0] - B[0]
                eng.tensor_sub(
                    eo[0:1, e0 + 4, :], eeoo[0:1, a0, :], eeoo[0:1, b0, :]
                )

            # ---- mirrors for the upper half (chunks c=4..7) ----
            # M layout: 0..3 EreM, 4..7 OreM, 8..11 OimM, 12..15 -EimM
            mir = qpool.tile([P, NPF, TS], BF16, tag="em")
            mirror_specs = []
            for i, c in enumerate(range(4, 8)):
                mirror_specs += [
                    (i, [(0, 7 - c), (1, 8 - c)]),               # EreM: Rrev,R00 on E
                    (4 + i, [(0, 8 + 7 - c), (1, 8 + 8 - c)]),   # OreM on O
                    (8 + i, [(0, 8 + 11 - c)] + ([(1, 8 + 12 - c)] if c > 4 else [])),
                    (12 + i, [(2, 11 - c)] + ([(3, 12 - c)] if c > 4 else [])),
                ]

            def emit_mirror(mslot, terms):
                pm = psM.tile([P, TS], F32, tag="psM")
                for ti, (rsel, ech) in enumerate(terms):
                    nc.tensor.matmul(
                        pm[:],
                        rr_sb[:, rsel, :],
                        eo[:, ech, :],
                        start=(ti == 0),
                        stop=(ti == len(terms) - 1),
                    )
                nc.scalar.copy(mir[:, mslot, :], pm[:])

            # ---- GW block one slab behind, mirror matmuls woven in so their
            # drains spread over the GW phase instead of bursting ----
            if qv_prev is not None:
                _gw_block(nc, tc, gw_sb, qv_prev, out, s - 1, opool, psG,
                          mirror_specs, emit_mirror)
            else:
                for mslot, terms in mirror_specs:
                    emit_mirror(mslot, terms)

            # ---- combine + scan + complex multiply, per frequency pair ----
            qv = qvpool.tile([P, NPF, TS], BF16, tag="qv")
            for j in range(8):
                X = Xpool.tile([P, 2, TS], BF16, tag="X")
                tA = tpool.tile([P, TS], BF16, tag="tA")
                if j <= 3:
                    nc.vector.scalar_tensor_tensor(
                        tA[:], eo[:, 8 + j, :], ca_sb[:, j : j + 1], eo[:, j, :],
                        ALU.mult, ALU.add,
                    )
                    nc.vector.scalar_tensor_tensor(
                        X[:, 0, :], eo[:, 12 + j, :], cb_sb[:, j : j + 1], tA[:],
                        ALU.mult, ALU.add,
                    )
                    tB = tpool.tile([P, TS], BF16, tag="tB")
                    nc.vector.scalar_tensor_tensor(
                        tB[:], eo[:, 12 + j, :], ca_sb[:, 8 + j : 9 + j], eo[:, 4 + j, :],
                        ALU.mult, ALU.add,
                    )
                    nc.vector.scalar_tensor_tensor(
                        X[:, 1, :], eo[:, 8 + j, :], cb_sb[:, 8 + j : 9 + j], tB[:],
                        ALU.mult, ALU.add,
                    )
                else:
                    i = j - 4
                    nc.vector.scalar_tensor_tensor(
                        tA[:], mir[:, 4 + i, :], ca_sb[:, j : j + 1], mir[:, i, :],
                        ALU.mult, ALU.add,
                    )
                    nc.vector.scalar_tensor_tensor(
                        X[:, 0, :], mir[:, 8 + i, :], cb_sb[:, j : j + 1], tA[:],
                        ALU.mult, ALU.add,
                    )
                    tB = tpool.tile([P, TS], BF16, tag="tB")
                    nc.vector.scalar_tensor_tensor(
                        tB[:], mir[:, 8 + i, :], ca_sb[:, 8 + j : 9 + j], mir[:, 12 + i, :],
                        ALU.mult, ALU.add,
                    )
                    nc.vector.scalar_tensor_tensor(
                        X[:, 1, :], mir[:, 4 + i, :], cb_sb[:, 8 + j : 9 + j], tB[:],
                        ALU.mult, ALU.add,
                    )
                if j == 0:
                    # X row 1024 (chunk 8, p0) is Nyquist: Re X[1024] = E[0] - O[0]
                    nc.vector.tensor_sub(
                        X[0:1, 1, :], eo[0:1, 0, :], eo[0:1, 8, :]
                    )

                S = spool.tile([P, 2, TS], BF16, tag="S")
                nc.vector.tensor_tensor_scan(
                    S[:, 0, :], X[:, 0, :], zeros[:], carry[:, j : j + 1],
                    ALU.add, ALU.add,
                )
                nc.vector.tensor_tensor_scan(
                    S[:, 1, :], X[:, 1, :], zeros[:], carry[:, j + 8 : j + 9],
                    ALU.add, ALU.add,
                )
                nc.vector.tensor_copy(carry[:, j : j + 1], S[:, 0, TS - 1 : TS])
                nc.vector.tensor_copy(carry[:, j + 8 : j + 9], S[:, 1, TS - 1 : TS])

                t1 = tpool.tile([P, TS], BF16, tag="t1")
                t2 = tpool.tile([P, TS], BF16, tag="t2")
                nc.vector.tensor_mul(t1[:], X[:, 0, :], S[:, 0, :])
                nc.vector.tensor_mul(t2[:], X[:, 1, :], S[:, 1, :])
                nc.vector.tensor_sub(qv[:, j, :], t1[:], t2[:])
                t3 = tpool.tile([P, TS], BF16, tag="t1")
                t4 = tpool.tile([P, TS], BF16, tag="t2")
                nc.vector.tensor_mul(t3[:], X[:, 0, :], S[:, 1, :])
                nc.vector.tensor_mul(t4[:], X[:, 1, :], S[:, 0, :])
                nc.vector.tensor_add(qv[:, j + 8, :], t3[:], t4[:])
                if j == 0:
                    # DC (chunk 0 p0) and Nyquist (chunk 8 p0) are purely real
                    nc.vector.tensor_mul(qv[0:1, 0, :], X[0:1, 0, :], S[0:1, 0, :])
                    nc.vector.tensor_mul(qv[0:1, 8, :], X[0:1, 1, :], S[0:1, 1, :])

            qv_prev = qv
        _gw_block(nc, tc, gw_sb, qv_prev, out, NSL - 1, opool, psG)


def _gw_block(nc, tc, gw_sb, qv, out, s, opool, psG, mirror_specs=None, emit_mirror=None):
    """out[s*TS + tg*128 + t, e] = sum_r qv[r, tg*128+t] * GW[r, e]"""
    for tg in range(TS // P):
        for eh in range(2):
            if mirror_specs:
                # L0 mirrors depend on the gpsimd L1-combine chain; weave them
                # into the later GW iterations so the PE never waits on it
                it = 2 * tg + eh
                if it in (5, 6):
                    for mslot, terms in mirror_specs[8 * (it - 5) : 8 * (it - 5) + 8]:
                        emit_mirror(mslot, terms)
            ps = psG.tile([P, 2, 512], F32, tag="psG")
            for pf in range(NPF):
                for e2 in range(2):
                    e = 2 * eh + e2
                    nc.tensor.matmul(
                        ps[:, e2, :],
                        qv[:, pf, tg * P : (tg + 1) * P],
                        gw_sb[:, pf, e * 512 : (e + 1) * 512],
                        start=(pf == 0),
                        stop=(pf == NPF - 1),
                    )
            r0 = s * TS + tg * P
            for e2 in range(2):
                osb = opool.tile([P, 512], F32, tag="osb")
                nc.scalar.copy(osb[:], ps[:, e2, :])
                e = 2 * eh + e2
                nc.sync.dma_start(out[r0 : r0 + P, e * 512 : (e + 1) * 512], osb[:])


def _chunked(m):
    """[rows, cols] -> [P, rows//P, cols] with row r at [r % P, r // P]."""
    r, c = m.shape
    return np.ascontiguousarray(m.reshape(r // P, P, c).transpose(1, 0, 2))


def _pack_spec(re, im):
    """re[1025], im[1025] -> packed [2048]: re[0..1024] then im[1..1023]."""
    return np.concatenate([re, im[1:1024]])


def _constants():
    if "consts" in _CACHE:
        return _CACHE["consts"]
    d = np.arange(D, dtype=np.float64)
    f = np.arange(D // 2 + 1, dtype=np.float64)
    ang = 2.0 * np.pi / D * np.outer(d, f)  # [D, 1025]
    cos, sin = np.cos(ang), np.sin(ang)
    alpha = np.full(1025, 2.0)
    alpha[0] = alpha[1024] = 1.0
    Gf = np.concatenate(
        [(alpha[:, None] * cos.T) / D, (-2.0 * sin[:, 1:1024].T) / D], axis=0
    )  # [D packed, D]

    # packed 512-point DFT matrix [512 rows m, 512 packed cols]
    m1 = np.arange(512, dtype=np.float64)
    q1 = np.arange(257, dtype=np.float64)
    ang1 = 2.0 * np.pi / 512 * np.outer(m1, q1)
    CS512 = np.concatenate(
        [np.cos(ang1), -np.sin(ang1)[:, 1:256]], axis=1
    )  # [512, 512]

    # mirror stationaries
    Rrev = np.zeros((P, P))
    for q in range(1, P):
        Rrev[q, P - q] = 1.0
    R00 = np.zeros((P, P))
    R00[0, 0] = 1.0
    RR = np.stack([Rrev, R00, -Rrev, -R00])  # [4, P, P] (lhsT: [K, M] per slot)

    # combine scalars for a radix-2 level producing a 2N-point spectrum with
    # 2h Re-chunks: CA/CB [P, 4h]
    p = np.arange(P, dtype=np.float64)

    def tables(h, twoN):
        CAm = np.zeros((P, 4 * h))
        CBm = np.zeros((P, 4 * h))
        for c in range(2 * h):  # Re side
            k = 128 * c + p
            CAm[:, c] = np.cos(2 * np.pi * k / twoN)
            CBm[:, c] = np.sin(2 * np.pi * k / twoN) * (1.0 if c < h else -1.0)
        for cc in range(2 * h):  # Im side
            k = 128 * cc + p
            CAm[:, 2 * h + cc] = np.cos(2 * np.pi * k / twoN) * (
                1.0 if cc < h else -1.0
            )
            CBm[:, 2 * h + cc] = -np.sin(2 * np.pi * k / twoN)
        return CAm.astype(np.float32), CBm.astype(np.float32)

    CAm, CBm = tables(4, D)
    CA1m, CB1m = tables(2, 1024)

    consts = {
        "CS": _chunked(CS512.astype(np.float32)).astype(bf16),  # [P, 4, 512]
        "RR": np.ascontiguousarray(RR.transpose(1, 0, 2)).astype(bf16),  # [P,4,P]
        "CA": CAm,
        "CB": CBm,
        "CA1": CA1m,
        "CB1": CB1m,
        "Gf": Gf,
    }
    _CACHE["consts"] = consts
    return consts


def prepare_in_maps(x, queries, keyvalues, w_out):
    x = np.asarray(x, dtype=np.float32)
    queries = np.asarray(queries, dtype=np.float32)
    keyvalues = np.asarray(keyvalues, dtype=np.float32)
    w_out = np.asarray(w_out, dtype=np.float32)
    consts = _constants()

    c = (queries * keyvalues).reshape(-1)  # [1025]
    c_packed = _pack_spec(c, c)  # [2048]
    GWf = (c_packed[:, None] * consts["Gf"]).astype(np.float32) @ np.ascontiguousarray(
        w_out.T
    )  # [D packed, D out]
    GWc = _chunked(GWf).astype(bf16)

    in_maps = []
    for b in range(NB):
        for h in range(2):
            xs = x[b, h * T : (h + 1) * T]  # [T, D]
            xcat = np.concatenate(
                [
                    _chunked(np.ascontiguousarray(xs[:, off::4].T))  # [P, 4, T]
                    for off in (0, 2, 1, 3)  # s00, s01, s10, s11
                ],
                axis=1,
            )  # [P, 16, T]
            xSc = np.ascontiguousarray(
                xcat.reshape(P, ND, NSL, TS).transpose(2, 0, 1, 3)
            ).astype(bf16)
            if h == 0:
                c0 = np.zeros((P, NPF), np.float32)
            else:
                F = np.fft.rfft(x[b, :T].sum(axis=0).astype(np.float64))
                c0 = _chunked(
                    _pack_spec(F.real, F.imag).astype(np.float32)[:, None]
                )[:, :, 0]
            in_maps.append(
                {
                    "xS": xSc,
                    "CS": consts["CS"],
                    "RR": consts["RR"],
                    "CA": consts["CA"],
                    "CB": consts["CB"],
                    "CA1": consts["CA1"],
                    "CB1": consts["CB1"],
                    "GW": GWc,
                    "C0": np.ascontiguousarray(c0),
                }
            )
    return in_maps


def kernel(x, queries, keyvalues, w_out):
    if "nc" not in _CACHE:
        _CACHE["nc"] = _build_nc()
    nc = _CACHE["nc"]
    in_maps = prepare_in_maps(x, queries, keyvalues, w_out)
    res = run_bass_kernel_spmd(nc, in_maps, core_ids=list(range(8)))
    y = np.empty((NB, NS, D), np.float32)
    for i in range(8):
        b, h = i // 2, i % 2
        y[b, h * T : (h + 1) * T] = res.results[i]["out"]
    return y


# revision 33
# speedup vs baseline: 1.3830x; 1.0100x over previous
"""HRR binding self-attention kernel for 8 trn2 NeuronCores.

Math: out = irfft(c * rfft(x) * cumsum_s(rfft(x))) @ w_out.T  with c = queries*keyvalues.
rfft is linear so cumsum commutes with it; the prefix sum runs in the frequency
domain.  irfft and the output Linear fuse into one matmul: out = qv^T @ GW with
GW = (c*G) @ w_out.T precomputed on host.

The forward rfft is radix-2 split: X[k] = E[k] + W^k O[k] with E,O = packed
rfft_1024 of even/odd samples — two 1024-wide DFT matmuls (half the PE work of
a direct 2048 DFT).  The upper half spectrum (k>512) needs partition-mirrored
E/O rows: cheap permutation matmuls (reversal + p0-pick stationaries) provide
them; conjugation signs and the W^k twiddle fold into per-partition scalars of
scalar_tensor_tensor combine ops on the DVE.

Packed spectra (1024 rows for E/O, 2048 for X): Re[0..N/2] then Im[1..N/2-1];
chunked [P=128 x chunks], chunk j pairs with chunk j+nchunks/2 on equal
partitions for complex ops; DC/Nyquist ride partition 0 with fixups.

Sharding: 8 shards = (batch b in 0..3) x (seq half h in 0..1), 2048 tokens
each; h=1 shards get the first half's spectrum sum as initial cumsum carry
(host rfft of x[b,:2048].sum(0)).

Per-core pipeline over 512-token slabs (all matmuls bf16, fp32 PSUM):
  EO-DFT -> ACT drains to SBUF -> mirror matmuls -> DVE combine -> DVE
  tensor_tensor_scan (cumsum, carry chained across slabs) -> DVE complex
  multiply -> qv bf16 -> GW matmul (one slab behind, keeps PE dense) ->
  ACT drain -> DMA out.
"""

import sys

sys.path.insert(0, "/opt/trn_rl_repo")

import numpy as np
import ml_dtypes

import concourse.bass as bass
import concourse.bacc as bacc
import concourse.mybir as mybir
from concourse.tile import TileContext
from concourse.bass_utils import run_bass_kernel_spmd

BF16 = mybir.dt.bfloat16
F32 = mybir.dt.float32
ALU = mybir.AluOpType

P = 128
D = 2048  # model dims
T = 2048  # tokens per shard
ND = D // P  # 16 chunks of x (8 even + 8 odd)
NPF = 16  # packed-frequency chunks of X
TS = 512  # tokens per slab
NSL = T // TS  # slabs
NB = 4  # batch
NS = 4096  # full seq

bf16 = ml_dtypes.bfloat16

_CACHE = {}


def _build_nc(reps: int = 1):
    nc = bacc.Bacc("TRN2", target_bir_lowering=False, debug=False, num_devices=8)
    # xS chunk 4s+q = sequence s (s00=x[0::4], s01=x[2::4], s10=x[1::4],
    # s11=x[3::4]) chunked by 128 rows
    xS = nc.dram_tensor("xS", [NSL, P, ND, TS], BF16, kind="ExternalInput")
    CS = nc.dram_tensor("CS", [P, 4, 512], BF16, kind="ExternalInput")  # 512-DFT
    RR = nc.dram_tensor("RR", [P, 4, P], BF16, kind="ExternalInput")  # Rrev,R00,-Rrev,-R00
    CA = nc.dram_tensor("CA", [P, NPF], F32, kind="ExternalInput")
    CB = nc.dram_tensor("CB", [P, NPF], F32, kind="ExternalInput")
    CA1 = nc.dram_tensor("CA1", [P, 8], F32, kind="ExternalInput")
    CB1 = nc.dram_tensor("CB1", [P, 8], F32, kind="ExternalInput")
    GW = nc.dram_tensor("GW", [P, NPF, D], BF16, kind="ExternalInput")
    C0 = nc.dram_tensor("C0", [P, NPF], F32, kind="ExternalInput")
    out = nc.dram_tensor("out", [T, D], F32, kind="ExternalOutput")

    with TileContext(nc) as tc:
        import contextlib

        loop_ctx = tc.For_i(0, reps, 1) if reps > 1 else contextlib.nullcontext()
        with loop_ctx:
            _body(nc, tc, xS, CS, RR, CA, CB, CA1, CB1, GW, C0, out)
    nc.finalize()
    return nc


def _body(nc, tc, xS, CS, RR, CA, CB, CA1, CB1, GW, C0, out):
    with (
        tc.tile_pool(name="const", bufs=1) as cpool,
        tc.tile_pool(name="xt", bufs=2) as xpool,
        tc.tile_pool(name="eeoo_mir", bufs=1) as qpool,
        tc.tile_pool(name="mirL1", bufs=1) as m1pool,
        tc.tile_pool(name="eo", bufs=1) as eopool,
        tc.tile_pool(name="X", bufs=1) as Xpool,
        tc.tile_pool(name="qv", bufs=2) as qvpool,
        tc.tile_pool(name="s", bufs=1) as spool,
        tc.tile_pool(name="tmp", bufs=1) as tpool,
        tc.tile_pool(name="gt", bufs=1) as gtpool,
        tc.tile_pool(name="osb", bufs=2) as opool,
        tc.tile_pool(name="psEO", bufs=2, space="PSUM") as psEO,
        tc.tile_pool(name="psM", bufs=2, space="PSUM") as psM,
        tc.tile_pool(name="psG", bufs=2, space="PSUM") as psG,
    ):
        # slab 0 input first on the ring so the first DFT starts ASAP
        xt0 = xpool.tile([P, ND, TS], BF16, tag="xt")
        nc.sync.dma_start(xt0[:], xS[0])
        cs_sb = cpool.tile([P, 4, 512], BF16)
        nc.sync.dma_start(cs_sb[:], CS[:])
        rr_sb = cpool.tile([P, 4, P], BF16)
        nc.sync.dma_start(rr_sb[:], RR[:])
        ca_sb = cpool.tile([P, NPF], F32)
        nc.sync.dma_start(ca_sb[:], CA[:])
        cb_sb = cpool.tile([P, NPF], F32)
        nc.sync.dma_start(cb_sb[:], CB[:])
        ca1_sb = cpool.tile([P, 8], F32)
        nc.sync.dma_start(ca1_sb[:], CA1[:])
        cb1_sb = cpool.tile([P, 8], F32)
        nc.sync.dma_start(cb1_sb[:], CB1[:])
        carry = cpool.tile([P, NPF], F32)
        nc.sync.dma_start(carry[:], C0[:])
        gw_sb = cpool.tile([P, NPF, D], BF16)
        for g in range(4):
            nc.sync.dma_start(
                gw_sb[:, 4 * g : 4 * (g + 1), :], GW[:, 4 * g : 4 * (g + 1), :]
            )
        zeros = cpool.tile([P, TS], BF16)
        nc.vector.memset(zeros[:], 0.0)

        qv_prev = None
        for s in range(NSL):
            if s == 0:
                xt = xt0
            else:
                xt = xpool.tile([P, ND, TS], BF16, tag="xt")
                nc.sync.dma_start(xt[:], xS[s])

            # ---- 512-DFT (16 packed chunks, 4 per sequence) with each pair's
            # L1 mirrors emitted as soon as its two sequences are drained, so
            # the DVE combine chain starts ~7us earlier ----
            eeoo = qpool.tile([P, NPF, TS], BF16, tag="em")
            mir1 = m1pool.tile([P, NPF, TS], BF16, tag="mir1")

            def emit_l1_mirrors(pr):
                a0, b0 = 8 * pr, 8 * pr + 4
                mb = 8 * pr
                l1specs = [
                    (mb + 0, [(0, a0 + 1), (1, a0 + 2)]),
                    (mb + 1, [(0, a0 + 0), (1, a0 + 1)]),
                    (mb + 2, [(0, b0 + 1), (1, b0 + 2)]),
                    (mb + 3, [(0, b0 + 0), (1, b0 + 1)]),
                    (mb + 4, [(0, b0 + 3)]),
                    (mb + 5, [(0, b0 + 2), (1, b0 + 3)]),
                    (mb + 6, [(2, a0 + 3)]),
                    (mb + 7, [(2, a0 + 2), (3, a0 + 3)]),
                ]
                for mslot, terms in l1specs:
                    pm = psM.tile([P, TS], F32, tag="psM")
                    for ti, (rsel, ech) in enumerate(terms):
                        nc.tensor.matmul(
                            pm[:],
                            rr_sb[:, rsel, :],
                            eeoo[:, ech, :],
                            start=(ti == 0),
                            stop=(ti == len(terms) - 1),
                        )
                    nc.scalar.copy(mir1[:, mslot, :], pm[:])

            for oc in range(NPF):
                base = 4 * (oc // 4)
                col = oc % 4
                ps = psEO.tile([P, TS], F32, tag="psEO")
                for d in range(4):
                    nc.tensor.matmul(
                        ps[:],
                        cs_sb[:, d, col * P : (col + 1) * P],
                        xt[:, base + d, :],
                        start=(d == 0),
                        stop=(d == 3),
                    )
                nc.scalar.copy(eeoo[:, oc, :], ps[:])
                if oc == 7:
                    emit_l1_mirrors(0)
            emit_l1_mirrors(1)

            # L1 combines on GPSIMD (PE and DVE stay free)
            eo = eopool.tile([P, NPF, TS], BF16, tag="eo")
            for pr in range(2):
                eng = nc.vector  # TensorScalarPtr is not legal on Pool/GPSIMD
                a0, b0, e0, mb = 8 * pr, 8 * pr + 4, 8 * pr, 8 * pr
                for c in range(4):  # Re side of the 1024-spectrum
                    gt = gtpool.tile([P, TS], BF16, tag=f"gt{pr}")
                    if c < 2:
                        eng.scalar_tensor_tensor(
                            gt[:], eeoo[:, b0 + c, :], ca1_sb[:, c : c + 1],
                            eeoo[:, a0 + c, :], ALU.mult, ALU.add,
                        )
                        eng.scalar_tensor_tensor(
                            eo[:, e0 + c, :], eeoo[:, b0 + 2 + c, :],
                            cb1_sb[:, c : c + 1], gt[:], ALU.mult, ALU.add,
                        )
                    else:
                        i = c - 2
                        eng.scalar_tensor_tensor(
                            gt[:], mir1[:, mb + 2 + i, :], ca1_sb[:, c : c + 1],
                            mir1[:, mb + i, :], ALU.mult, ALU.add,
                        )
                        eng.scalar_tensor_tensor(
                            eo[:, e0 + c, :], mir1[:, mb + 4 + i, :],
                            cb1_sb[:, c : c + 1], gt[:], ALU.mult, ALU.add,
                        )
                for cc in range(4):  # Im side
                    gt = gtpool.tile([P, TS], BF16, tag=f"gt{pr}")
                    if cc < 2:
                        eng.scalar_tensor_tensor(
                            gt[:], eeoo[:, b0 + 2 + cc, :], ca1_sb[:, 4 + cc : 5 + cc],
                            eeoo[:, a0 + 2 + cc, :], ALU.mult, ALU.add,
                        )
                        eng.scalar_tensor_tensor(
                            eo[:, e0 + 4 + cc, :], eeoo[:, b0 + cc, :],
                            cb1_sb[:, 4 + cc : 5 + cc], gt[:], ALU.mult, ALU.add,
                        )
                    else:
                        i = cc - 2
                        eng.scalar_tensor_tensor(
                            gt[:], mir1[:, mb + 4 + i, :], ca1_sb[:, 4 + cc : 5 + cc],
                            mir1[:, mb + 6 + i, :], ALU.mult, ALU.add,
                        )
                        eng.scalar_tensor_tensor(
                            eo[:, e0 + 4 + cc, :], mir1[:, mb + 2 + i, :],
                            cb1_sb[:, 4 + cc : 5 + cc], gt[:], ALU.mult, ALU.add,
                        )
                # 1024-Nyquist row (chunk e0+4, p0): A[0] - B[0]
                eng.tensor_sub(
                    eo[0:1, e0 + 4, :], eeoo[0:1, a0, :], eeoo[0:1, b0, :]
                )

            # ---- mirrors for the upper half (chunks c=4..7) ----
            # M layout: 0..3 EreM, 4..7 OreM, 8..11 OimM, 12..15 -EimM
            mir = qpool.tile([P, NPF, TS], BF16, tag="em")
            mirror_specs = []
            for i, c in enumerate(range(4, 8)):
                mirror_specs += [
                    (i, [(0, 7 - c), (1, 8 - c)]),               # EreM: Rrev,R00 on E
                    (4 + i, [(0, 8 + 7 - c), (1, 8 + 8 - c)]),   # OreM on O
                    (8 + i, [(0, 8 + 11 - c)] + ([(1, 8 + 12 - c)] if c > 4 else [])),
                    (12 + i, [(2, 11 - c)] + ([(3, 12 - c)] if c > 4 else [])),
                ]

            def emit_mirror(mslot, terms):
                pm = psM.tile([P, TS], F32, tag="psM")
                for ti, (rsel, ech) in enumerate(terms):
                    nc.tensor.matmul(
                        pm[:],
                        rr_sb[:, rsel, :],
                        eo[:, ech, :],
                        start=(ti == 0),
                        stop=(ti == len(terms) - 1),
                    )
                nc.scalar.copy(mir[:, mslot, :], pm[:])

            # ---- GW block one slab behind, mirror matmuls woven in so their
            # drains spread over the GW phase instead of bursting ----
            if qv_prev is not None:
                _gw_block(nc, tc, gw_sb, qv_prev, out, s - 1, opool, psG,
                          mirror_specs, emit_mirror)
            else:
                for mslot, terms in mirror_specs:
                    emit_mirror(mslot, terms)

            # ---- combine + scan + complex multiply, per frequency pair ----
            qv = qvpool.tile([P, NPF, TS], BF16, tag="qv")
            for j in range(8):
                X = Xpool.tile([P, 2, TS], BF16, tag="X")
                tA = tpool.tile([P, TS], BF16, tag="tA")
                if j <= 3:
                    nc.vector.scalar_tensor_tensor(
                        tA[:], eo[:, 8 + j, :], ca_sb[:, j : j + 1], eo[:, j, :],
                        ALU.mult, ALU.add,
                    )
                    nc.vector.scalar_tensor_tensor(
                        X[:, 0, :], eo[:, 12 + j, :], cb_sb[:, j : j + 1], tA[:],
                        ALU.mult, ALU.add,
                    )
                    tB = tpool.tile([P, TS], BF16, tag="tB")
                    nc.vector.scalar_tensor_tensor(
                        tB[:], eo[:, 12 + j, :], ca_sb[:, 8 + j : 9 + j], eo[:, 4 + j, :],
                        ALU.mult, ALU.add,
                    )
                    nc.vector.scalar_tensor_tensor(
                        X[:, 1, :], eo[:, 8 + j, :], cb_sb[:, 8 + j : 9 + j], tB[:],
                        ALU.mult, ALU.add,
                    )
                else:
                    i = j - 4
                    nc.vector.scalar_tensor_tensor(
                        tA[:], mir[:, 4 + i, :], ca_sb[:, j : j + 1], mir[:, i, :],
                        ALU.mult, ALU.add,
                    )
                    nc.vector.scalar_tensor_tensor(
                        X[:, 0, :], mir[:, 8 + i, :], cb_sb[:, j : j + 1], tA[:],
                        ALU.mult, ALU.add,
                    )
                    tB = tpool.tile([P, TS], BF16, tag="tB")
                    nc.vector.scalar_tensor_tensor(
                        tB[:], mir[:, 8 + i, :], ca_sb[:, 8 + j : 9 + j], mir[:, 12 + i, :],
                        ALU.mult, ALU.add,
                    )
                    nc.vector.scalar_tensor_tensor(
                        X[:, 1, :], mir[:, 4 + i, :], cb_sb[:, 8 + j : 9 + j], tB[:],
                        ALU.mult, ALU.add,
                    )
                if j == 0:
                    # X row 1024 (chunk 8, p0) is Nyquist: Re X[1024] = E[0] - O[0]
                    nc.vector.tensor_sub(
                        X[0:1, 1, :], eo[0:1, 0, :], eo[0:1, 8, :]
                    )

                S = spool.tile([P, 2, TS], BF16, tag="S")
                nc.vector.tensor_tensor_scan(
                    S[:, 0, :], X[:, 0, :], zeros[:], carry[:, j : j + 1],
                    ALU.add, ALU.add,
                )
                nc.vector.tensor_tensor_scan(
                    S[:, 1, :], X[:, 1, :], zeros[:], carry[:, j + 8 : j + 9],
                    ALU.add, ALU.add,
                )
                nc.vector.tensor_copy(carry[:, j : j + 1], S[:, 0, TS - 1 : TS])
                nc.vector.tensor_copy(carry[:, j + 8 : j + 9], S[:, 1, TS - 1 : TS])

                t1 = tpool.tile([P, TS], BF16, tag="t1")
                t2 = tpool.tile([P, TS], BF16, tag="t2")
                nc.vector.tensor_mul(t1[:], X[:, 0, :], S[:, 0, :])
                nc.vector.tensor_mul(t2[:], X[:, 1, :], S[:, 1, :])
                nc.vector.tensor_sub(qv[:, j, :], t1[:], t2[:])
                t3 = tpool.tile([P, TS], BF16, tag="t1")
                t4 = tpool.tile([P, TS], BF16, tag="t2")
                nc.vector.tensor_mul(t3[:], X[:, 0, :], S[:, 1, :])
                nc.vector.tensor_mul(t4[:], X[:, 1, :], S[:, 0, :])
                nc.vector.tensor_add(qv[:, j + 8, :], t3[:], t4[:])
                if j == 0:
                    # DC (chunk 0 p0) and Nyquist (chunk 8 p0) are purely real
                    nc.vector.tensor_mul(qv[0:1, 0, :], X[0:1, 0, :], S[0:1, 0, :])
                    nc.vector.tensor_mul(qv[0:1, 8, :], X[0:1, 1, :], S[0:1, 1, :])

            qv_prev = qv
        _gw_block(nc, tc, gw_sb, qv_prev, out, NSL - 1, opool, psG)


def _gw_block(nc, tc, gw_sb, qv, out, s, opool, psG, mirror_specs=None, emit_mirror=None):
    """out[s*TS + tg*128 + t, e] = sum_r qv[r, tg*128+t] * GW[r, e]"""
    for tg in range(TS // P):
        for eh in range(2):
            if mirror_specs:
                # L0 mirrors depend on the gpsimd L1-combine chain; weave them
                # into the later GW iterations so the PE never waits on it
                it = 2 * tg + eh
                if it in (5, 6):
                    for mslot, terms in mirror_specs[8 * (it - 5) : 8 * (it - 5) + 8]:
                        emit_mirror(mslot, terms)
            ps = psG.tile([P, 2, 512], F32, tag="psG")
            for pf in range(NPF):
                for e2 in range(2):
                    e = 2 * eh + e2
                    nc.tensor.matmul(
                        ps[:, e2, :],
                        qv[:, pf, tg * P : (tg + 1) * P],
                        gw_sb[:, pf, e * 512 : (e + 1) * 512],
                        start=(pf == 0),
                        stop=(pf == NPF - 1),
                    )
            r0 = s * TS + tg * P
            for e2 in range(2):
                osb = opool.tile([P, 512], F32, tag="osb")
                nc.scalar.copy(osb[:], ps[:, e2, :])
                e = 2 * eh + e2
                nc.sync.dma_start(out[r0 : r0 + P, e * 512 : (e + 1) * 512], osb[:])


def _chunked(m):
    """[rows, cols] -> [P, rows//P, cols] with row r at [r % P, r // P]."""
    r, c = m.shape
    return np.ascontiguousarray(m.reshape(r // P, P, c).transpose(1, 0, 2))


def _pack_spec(re, im):
    """re[1025], im[1025] -> packed [2048]: re[0..1024] then im[1..1023]."""
    return np.concatenate([re, im[1:1024]])


def _constants():
    if "consts" in _CACHE:
        return _CACHE["consts"]
    d = np.arange(D, dtype=np.float64)
    f = np.arange(D // 2 + 1, dtype=np.float64)
    ang = 2.0 * np.pi / D * np.outer(d, f)  # [D, 1025]
    cos, sin = np.cos(ang), np.sin(ang)
    alpha = np.full(1025, 2.0)
    alpha[0] = alpha[1024] = 1.0
    Gf = np.concatenate(
        [(alpha[:, None] * cos.T) / D, (-2.0 * sin[:, 1:1024].T) / D], axis=0
    )  # [D packed, D]

    # packed 512-point DFT matrix [512 rows m, 512 packed cols]
    m1 = np.arange(512, dtype=np.float64)
    q1 = np.arange(257, dtype=np.float64)
    ang1 = 2.0 * np.pi / 512 * np.outer(m1, q1)
    CS512 = np.concatenate(
        [np.cos(ang1), -np.sin(ang1)[:, 1:256]], axis=1
    )  # [512, 512]

    # mirror stationaries
    Rrev = np.zeros((P, P))
    for q in range(1, P):
        Rrev[q, P - q] = 1.0
    R00 = np.zeros((P, P))
    R00[0, 0] = 1.0
    RR = np.stack([Rrev, R00, -Rrev, -R00])  # [4, P, P] (lhsT: [K, M] per slot)

    # combine scalars for a radix-2 level producing a 2N-point spectrum with
    # 2h Re-chunks: CA/CB [P, 4h]
    p = np.arange(P, dtype=np.float64)

    def tables(h, twoN):
        CAm = np.zeros((P, 4 * h))
        CBm = np.zeros((P, 4 * h))
        for c in range(2 * h):  # Re side
            k = 128 * c + p
            CAm[:, c] = np.cos(2 * np.pi * k / twoN)
            CBm[:, c] = np.sin(2 * np.pi * k / twoN) * (1.0 if c < h else -1.0)
        for cc in range(2 * h):  # Im side
            k = 128 * cc + p
            CAm[:, 2 * h + cc] = np.cos(2 * np.pi * k / twoN) * (
                1.0 if cc < h else -1.0
            )
            CBm[:, 2 * h + cc] = -np.sin(2 * np.pi * k / twoN)
        return CAm.astype(np.float32), CBm.astype(np.float32)

    CAm, CBm = tables(4, D)
    CA1m, CB1m = tables(2, 1024)

    consts = {
        "CS": _chunked(CS512.astype(np.float32)).astype(bf16),  # [P, 4, 512]
        "RR": np.ascontiguousarray(RR.transpose(1, 0, 2)).astype(bf16),  # [P,4,P]
        "CA": CAm,
        "CB": CBm,
        "CA1": CA1m,
        "CB1": CB1m,
        "Gf": Gf,
    }
    _CACHE["consts"] = consts
    return consts


def prepare_in_maps(x, queries, keyvalues, w_out):
    x = np.asarray(x, dtype=np.float32)
    queries = np.asarray(queries, dtype=np.float32)
    keyvalues = np.asarray(keyvalues, dtype=np.float32)
    w_out = np.asarray(w_out, dtype=np.float32)
    consts = _constants()

    c = (queries * keyvalues).reshape(-1)  # [1025]
    c_packed = _pack_spec(c, c)  # [2048]
    GWf = (c_packed[:, None] * consts["Gf"]).astype(np.float32) @ np.ascontiguousarray(
        w_out.T
    )  # [D packed, D out]
    GWc = _chunked(GWf).astype(bf16)

    in_maps = []
    for b in range(NB):
        for h in range(2):
            xs = x[b, h * T : (h + 1) * T]  # [T, D]
            xcat = np.concatenate(
                [
                    _chunked(np.ascontiguousarray(xs[:, off::4].T))  # [P, 4, T]
                    for off in (0, 2, 1, 3)  # s00, s01, s10, s11
                ],
                axis=1,
            )  # [P, 16, T]
            xSc = np.ascontiguousarray(
                xcat.reshape(P, ND, NSL, TS).transpose(2, 0, 1, 3)
            ).astype(bf16)
            if h == 0:
                c0 = np.zeros((P, NPF), np.float32)
            else:
                F = np.fft.rfft(x[b, :T].sum(axis=0).astype(np.float64))
                c0 = _chunked(
                    _pack_spec(F.real, F.imag).astype(np.float32)[:, None]
                )[:, :, 0]
            in_maps.append(
                {
                    "xS": xSc,
                    "CS": consts["CS"],
                    "RR": consts["RR"],
                    "CA": consts["CA"],
                    "CB": consts["CB"],
                    "CA1": consts["CA1"],
                    "CB1": consts["CB1"],
                    "GW": GWc,
                    "C0": np.ascontiguousarray(c0),
                }
            )
    return in_maps


def kernel(x, queries, keyvalues, w_out):
    if "nc" not in _CACHE:
        _CACHE["nc"] = _build_nc()
    nc = _CACHE["nc"]
    in_maps = prepare_in_maps(x, queries, keyvalues, w_out)
    res = run_bass_kernel_spmd(nc, in_maps, core_ids=list(range(8)))
    y = np.empty((NB, NS, D), np.float32)
    for i in range(8):
        b, h = i // 2, i % 2
        y[b, h * T : (h + 1) * T] = res.results[i]["out"]
    return y
